# revision 1
# baseline (speedup 1.0000x reference)
"""CortexIIBlock TRN2 Bass kernel v2 — fused per-block mixer+FFN pipeline.

8-core data-parallel over (batch, seq-half): each core owns 2048 sequence
positions (+16 history cols for the causal convs). All matmuls bf16 inputs
with fp32 PSUM accumulation. Per-block software pipeline: FFN matmuls of
block i-1 run on PE while DVE computes the depthwise convs of block i+1
and GpSimd does the softmax-weighted conv mixing. x2 residual stays in
SBUF (no DRAM round trip).
"""
import numpy as np

D = 1024
DFF = 4096
B = 4
S = 4096
H = 16           # history cols per shard (conv lookback <= 6, padded to 16)
R = 2048         # payload cols per shard
NCT = D // 128   # 8 channel tiles
NB = 4           # payload blocks
BN = 512
EPS = 1e-6

_CACHE = {}


def _build():
    import concourse.bacc as bacc
    import concourse.mybir as mybir
    import concourse.tile as tile

    F32 = mybir.dt.float32
    BF16 = mybir.dt.bfloat16
    AF = mybir.ActivationFunctionType
    MUL = mybir.AluOpType.mult
    ADD = mybir.AluOpType.add

    nc = bacc.Bacc(None, target_bir_lowering=False)
    _lp = nc.allow_low_precision(reason="bf16 matmuls/activations within tolerance")
    _lp.__enter__()

    xT_d = nc.dram_tensor("xT", [128, NCT, H + R], BF16, kind="ExternalInput")
    # grouped weights: pairs of m-tiles side by side [128, 2*D]
    up_d = nc.dram_tensor("up_g", [8, 128, 2 * D], BF16, kind="ExternalInput")     # g0..3 gate, g4..7 val
    down_d = nc.dram_tensor("down_g", [4, 128, 2 * D], BF16, kind="ExternalInput")
    wg_d = nc.dram_tensor("wg_g", [16, 128, 2 * D], BF16, kind="ExternalInput")
    wu_d = nc.dram_tensor("wu_g", [16, 128, 2 * D], BF16, kind="ExternalInput")
    wo_d = nc.dram_tensor("wo_sb", [8, 128, DFF], BF16, kind="ExternalInput")
    sg_d = nc.dram_tensor("sg_p", [128, NCT, 3], BF16, kind="ExternalInput")
    ln1_d = nc.dram_tensor("ln1_p", [128, NCT], F32, kind="ExternalInput")
    ln2_d = nc.dram_tensor("ln2_p", [128, NCT], F32, kind="ExternalInput")
    taps_d = nc.dram_tensor("taps_p", [128, NCT, 15], F32, kind="ExternalInput")
    ones128_d = nc.dram_tensor("ones128", [128, 1], BF16, kind="ExternalInput")
    one1_d = nc.dram_tensor("one1", [1, 128], BF16, kind="ExternalInput")
    yT_d = nc.dram_tensor("yT", [128, NCT, R], F32, kind="ExternalOutput")

    from contextlib import ExitStack
    with tile.TileContext(nc) as tc:
        with ExitStack() as stack:
            ep = stack.enter_context
            cpool = ep(tc.tile_pool(name="const", bufs=1))
            xp = ep(tc.tile_pool(name="xp", bufs=3))
            hp = ep(tc.tile_pool(name="hp", bufs=2))
            vp = ep(tc.tile_pool(name="vp", bufs=2))
            x2p = ep(tc.tile_pool(name="x2p", bufs=1))
            h2p = ep(tc.tile_pool(name="h2p", bufs=1))
            gp = ep(tc.tile_pool(name="gp", bufs=1))
            cvp = ep(tc.tile_pool(name="cvp", bufs=2))
            zp = ep(tc.tile_pool(name="zp", bufs=1))
            ppool = ep(tc.tile_pool(name="pp", bufs=1))
            sbp = ep(tc.tile_pool(name="sb", bufs=2))
            smp = ep(tc.tile_pool(name="sm", bufs=2))
            tgp = ep(tc.tile_pool(name="tg", bufs=3))
            yp = ep(tc.tile_pool(name="yp", bufs=2))
            sqp = ep(tc.tile_pool(name="sqq", bufs=3))
            wmx = ep(tc.tile_pool(name="wmix", bufs=4))
            wgup = ep(tc.tile_pool(name="wgu", bufs=2))
            wop = ep(tc.tile_pool(name="wop", bufs=3))
            psmm = ep(tc.tile_pool(name="psmm", bufs=4, space="PSUM"))
            psbc = ep(tc.tile_pool(name="psbc", bufs=2, space="PSUM"))
            psrd = ep(tc.tile_pool(name="psrd", bufs=2, space="PSUM"))
            # ---------------- constants ----------------
            ones128 = cpool.tile([128, 1], BF16, tag="c_ones", name="c_ones")
            nc.sync.dma_start(ones128[:], ones128_d[:])
            one1 = cpool.tile([1, 128], BF16, tag="c_one1", name="c_one1")
            nc.sync.dma_start(one1[:], one1_d[:])
            eps_t = cpool.tile([1, 1], F32, tag="c_eps", name="c_eps")
            nc.vector.memset(eps_t[:], EPS)
            xh = cpool.tile([128, NCT, H], BF16, tag="xh", name="xh")
            nc.sync.dma_start(xh[:], xT_d[:, :, 0:H])
            ln1_t = cpool.tile([128, NCT], F32, tag="c_ln1", name="c_ln1")
            nc.sync.dma_start(ln1_t[:], ln1_d[:])

            # ---------------- persistent per-block state ----------------
            xb = [None] * NB
            hT = [None] * NB
            val = [None] * NB
            gate = [None] * NB
            swb = [None] * NB
            z = [None] * NB
            x2 = [None] * NB
            h2 = [None] * NB
            pT = [None] * NB

            def rmsnorm(src, ln_t, tag, is_mixer):
                msum = psrd.tile([1, BN], F32, tag="msum", name=f"msum_{tag}")
                for c in range(NCT):
                    sq = sqp.tile([128, BN], BF16, tag="sq", name=f"sq_{tag}{c}")
                    nc.scalar.activation(sq[:], src[:, c, :], AF.Square)
                    nc.tensor.matmul(msum[:], ones128[:], sq[:],
                                     start=(c == 0), stop=(c == NCT - 1))
                # rstd = exp(-0.5*ln(ms/D + eps)) — all on Act, no DVE dependency
                sd = smp.tile([1, BN], F32, tag="sd", name=f"sd_{tag}")
                nc.scalar.activation(sd[:], msum[:], AF.Ln,
                                     bias=eps_t[:], scale=1.0 / D)
                rstd = smp.tile([1, BN], BF16, tag="rstd", name=f"rstd_{tag}")
                nc.scalar.activation(rstd[:], sd[:], AF.Exp, scale=-0.5)
                rsb_ps = psbc.tile([128, BN], F32, tag="pbc", name=f"rsbp_{tag}")
                nc.tensor.matmul(rsb_ps[:], one1[:], rstd[:], start=True, stop=True)
                rsb = sbp.tile([128, BN], BF16, tag="rsb", name=f"rsb_{tag}")
                nc.scalar.copy(rsb[:], rsb_ps[:])
                h_ = (hp if is_mixer else h2p).tile(
                    [128, NCT, BN], BF16, tag="h" if is_mixer else "h2",
                    name=f"h_{tag}")
                for c in range(NCT):
                    nc.vector.scalar_tensor_tensor(
                        out=h_[:, c, :], in0=src[:, c, :],
                        scalar=ln_t[:, c:c + 1], in1=rsb[:], op0=MUL, op1=MUL)
                return h_

            # ---------------- history mini-front (16 cols) ----------------
            msumh = psrd.tile([1, H], F32, tag="msum", name="msumh")
            for c in range(NCT):
                sqh = sqp.tile([128, H], BF16, tag="sqh", name=f"sqh{c}", bufs=2)
                nc.scalar.activation(sqh[:], xh[:, c, :], AF.Square)
                nc.tensor.matmul(msumh[:], ones128[:], sqh[:],
                                 start=(c == 0), stop=(c == NCT - 1))
            sdh = smp.tile([1, H], F32, tag="sdh", name="sdh", bufs=1)
            nc.scalar.activation(sdh[:], msumh[:], AF.Ln, bias=eps_t[:], scale=1.0 / D)
            rstdh = smp.tile([1, H], BF16, tag="rstdh", name="rstdh", bufs=1)
            nc.scalar.activation(rstdh[:], sdh[:], AF.Exp, scale=-0.5)
            rsbh_ps = psbc.tile([128, H], F32, tag="pbc", name="rsbph")
            nc.tensor.matmul(rsbh_ps[:], one1[:], rstdh[:], start=True, stop=True)
            rsbh = smp.tile([128, H], BF16, tag="rsbh", name="rsbh", bufs=1)
            nc.scalar.copy(rsbh[:], rsbh_ps[:])
            hh = cpool.tile([128, NCT, H], BF16, tag="hh", name="hh")
            for c in range(NCT):
                nc.vector.scalar_tensor_tensor(
                    out=hh[:, c, :], in0=xh[:, c, :],
                    scalar=ln1_t[:, c:c + 1], in1=rsbh[:], op0=MUL, op1=MUL)
            valh = []
            for g in range(4):
                wt = wmx.tile([128, 2 * D], BF16, tag="wmix", name=f"wvh{g}")
                nc.sync.dma_start(wt[:], up_d[4 + g])
                for j in range(2):
                    m = 2 * g + j
                    pvh = psbc.tile([128, H], F32, tag="pbc", name=f"pvh{m}")
                    for k in range(NCT):
                        nc.tensor.matmul(pvh[:], wt[:, j * D + k * 128:j * D + (k + 1) * 128],
                                         hh[:, k, :], start=(k == 0), stop=(k == NCT - 1))
                    vh = cpool.tile([128, H], BF16, tag=f"vh{m}", name=f"vh{m}")
                    nc.scalar.copy(vh[:], pvh[:])
                    valh.append(vh)

            sg_t = cpool.tile([128, NCT, 3], BF16, tag="c_sg", name="c_sg")
            nc.sync.dma_start(sg_t[:], sg_d[:])
            ln2_t = cpool.tile([128, NCT], F32, tag="c_ln2", name="c_ln2")
            nc.sync.dma_start(ln2_t[:], ln2_d[:])
            taps_t = cpool.tile([128, NCT, 15], F32, tag="c_taps", name="c_taps")
            nc.sync.dma_start(taps_t[:], taps_d[:])

            # ---------------- per-block pieces ----------------
            def norm_front(i):
                c0 = H + i * BN
                x_ = xp.tile([128, NCT, BN], BF16, tag="xb", name=f"xb{i}")
                nc.sync.dma_start(x_[:], xT_d[:, :, c0:c0 + BN])
                xb[i] = x_
                hT[i] = rmsnorm(x_, ln1_t, f"m{i}", True)

            def body_front(i):
                # scale gates: softmax over 3 per-row chains (partition offsets
                # must be multiples of 32, so keep each row at partition 0)
                ej = []
                for j in range(3):
                    pj = psrd.tile([1, BN], F32, tag="msum", name=f"psg{i}_{j}")
                    for k in range(NCT):
                        nc.tensor.matmul(pj[:], sg_t[:, k, j:j + 1], hT[i][:, k, :],
                                         start=(k == 0), stop=(k == NCT - 1))
                    e_ = smp.tile([1, BN], BF16, tag=f"e{j}", name=f"e{i}_{j}", bufs=1)
                    nc.scalar.activation(e_[:], pj[:], AF.Exp)
                    ej.append(e_)
                es = smp.tile([1, BN], BF16, tag="es", name=f"es{i}")
                nc.vector.tensor_add(es[:], ej[0][:], ej[1][:])
                nc.vector.tensor_add(es[:], es[:], ej[2][:])
                erec = smp.tile([1, BN], BF16, tag="erec", name=f"erec{i}")
                nc.vector.reciprocal(erec[:], es[:])
                sw_ = []
                for j in range(3):
                    swj = smp.tile([1, BN], BF16, tag="swj", name=f"swj{i}_{j}")
                    nc.vector.tensor_mul(swj[:], ej[j][:], erec[:])
                    pb_ = psbc.tile([128, BN], F32, tag="pbc", name=f"pswb{i}_{j}")
                    nc.tensor.matmul(pb_[:], one1[:], swj[:], start=True, stop=True)
                    sb_ = sbp.tile([128, BN], BF16, tag=f"swb{j}", name=f"swb{i}_{j}", bufs=1)
                    nc.scalar.copy(sb_[:], pb_[:])
                    sw_.append(sb_)
                swb[i] = sw_

                # val half of up projection (groups 4..7)
                vtiles = []
                for m in range(NCT):
                    v_ = vp.tile([128, H + BN], BF16, tag=f"val{m}", name=f"val{i}_{m}")
                    vtiles.append(v_)
                val[i] = vtiles
                for g in range(4):
                    wt = wmx.tile([128, 2 * D], BF16, tag="wmix", name=f"wv{i}_{g}")
                    nc.sync.dma_start(wt[:], up_d[4 + g])
                    for j in range(2):
                        m = 2 * g + j
                        pv = psmm.tile([128, BN], F32, tag="pmm", name=f"pval{i}_{m}")
                        for k in range(NCT):
                            nc.tensor.matmul(pv[:], wt[:, j * D + k * 128:j * D + (k + 1) * 128],
                                             hT[i][:, k, :], start=(k == 0), stop=(k == NCT - 1))
                        nc.scalar.copy(vtiles[m][:, H:H + BN], pv[:])
                        if i == 0:
                            nc.vector.tensor_copy(vtiles[m][:, 0:H], valh[m][:])
                        else:
                            nc.vector.tensor_copy(vtiles[m][:, 0:H], val[i - 1][m][:, BN:BN + H])

                # gate half of up projection (groups 0..3)
                g_ = gp.tile([128, NCT, BN], BF16, tag="gate", name=f"gate{i}")
                gate[i] = g_
                for g in range(4):
                    wt = wmx.tile([128, 2 * D], BF16, tag="wmix", name=f"wgm{i}_{g}")
                    nc.sync.dma_start(wt[:], up_d[g])
                    for j in range(2):
                        m = 2 * g + j
                        pg = psmm.tile([128, BN], F32, tag="pmm", name=f"pgate{i}_{m}")
                        for k in range(NCT):
                            nc.tensor.matmul(pg[:], wt[:, j * D + k * 128:j * D + (k + 1) * 128],
                                             hT[i][:, k, :], start=(k == 0), stop=(k == NCT - 1))
                        nc.scalar.activation(g_[:, m, :], pg[:], AF.Sigmoid)

                # convs + softmax-weighted mix.
                # Steady state: convs on DVE, mix on GpSimd, both at low priority
                # (pure gap-filler; deadline is down(i) one iteration later).
                # Block 0 is the pipeline prologue and on the critical path, so
                # split the work across both engines at normal priority instead.
                z_ = zp.tile([128, NCT, BN], BF16, tag="z", name=f"z{i}")
                z[i] = z_
                lowp = None
                if i > 0:
                    lowp = tc.high_priority(offset=-10_000_000)
                    lowp.__enter__()
                for c in range(NCT):
                    conv_eng = nc.vector
                    mix_eng = nc.vector if (i == 0 and c >= 6) else nc.gpsimd
                    v_ = vtiles[c]
                    convs = []
                    for (nt, base) in ((3, 0), (5, 3), (7, 8)):
                        ct_ = cvp.tile([128, BN], BF16, tag=f"cv{len(convs)}",
                                       name=f"cv{i}_{c}_{len(convs)}")
                        conv_eng.tensor_scalar_mul(
                            ct_[:], v_[:, H:H + BN], taps_t[:, c, base:base + 1])
                        for j in range(1, nt):
                            conv_eng.scalar_tensor_tensor(
                                out=ct_[:], in0=v_[:, H - j:H - j + BN],
                                scalar=taps_t[:, c, base + j:base + j + 1],
                                in1=ct_[:], op0=MUL, op1=ADD)
                        convs.append(ct_)
                    acc = cvp.tile([128, BN], BF16, tag="acc", name=f"acc{i}_{c}")
                    mix_eng.tensor_mul(acc[:], convs[0][:], sw_[0][:])
                    for j in (1, 2):
                        u_ = cvp.tile([128, BN], BF16, tag="mixu", name=f"mixu{i}_{c}")
                        mix_eng.tensor_mul(u_[:], convs[j][:], sw_[j][:])
                        mix_eng.tensor_add(acc[:], acc[:], u_[:])
                    mix_eng.tensor_mul(z_[:, c, :], acc[:], g_[:, c, :])
                if lowp is not None:
                    lowp.__exit__(None, None, None)

            def down_block(i):
                x2_ = x2p.tile([128, NCT, BN], BF16, tag="x2", name=f"x2_{i}")
                x2[i] = x2_
                for g in range(4):
                    wt = wmx.tile([128, 2 * D], BF16, tag="wmix", name=f"wd{i}_{g}")
                    nc.sync.dma_start(wt[:], down_d[g])
                    for j in range(2):
                        m = 2 * g + j
                        pm = psmm.tile([128, BN], F32, tag="pmm", name=f"pmix{i}_{m}")
                        for k in range(NCT):
                            nc.tensor.matmul(pm[:], wt[:, j * D + k * 128:j * D + (k + 1) * 128],
                                             z[i][:, k, :], start=(k == 0), stop=(k == NCT - 1))
                        nc.vector.tensor_add(x2_[:, m, :], xb[i][:, m, :], pm[:])

            def ffn_norm(i):
                h2[i] = rmsnorm(x2[i], ln2_t, f"f{i}", False)

            def ffn_gup(i):
                p_ = []
                for g in range(16):
                    wtg = wgup.tile([128, 2 * D], BF16, tag="wg", name=f"wgt{i}_{g}")
                    nc.sync.dma_start(wtg[:], wg_d[g])
                    wtu = wgup.tile([128, 2 * D], BF16, tag="wu", name=f"wut{i}_{g}")
                    nc.sync.dma_start(wtu[:], wu_d[g])
                    for j in range(2):
                        m = 2 * g + j
                        pg = psmm.tile([128, BN], F32, tag="pmm", name=f"pg{i}_{m}")
                        for k in range(NCT):
                            nc.tensor.matmul(pg[:], wtg[:, j * D + k * 128:j * D + (k + 1) * 128],
                                             h2[i][:, k, :], start=(k == 0), stop=(k == NCT - 1))
                        pu = psmm.tile([128, BN], F32, tag="pmm", name=f"pu{i}_{m}")
                        for k in range(NCT):
                            nc.tensor.matmul(pu[:], wtu[:, j * D + k * 128:j * D + (k + 1) * 128],
                                             h2[i][:, k, :], start=(k == 0), stop=(k == NCT - 1))
                        tg = tgp.tile([128, BN], BF16, tag="tg", name=f"tg{i}_{m}")
                        nc.scalar.activation(tg[:], pg[:], AF.Silu)
                        pt = ppool.tile([128, BN], BF16, tag=f"p{m}", name=f"p{i}_{m}")
                        nc.vector.tensor_mul(pt[:], tg[:], pu[:])
                        p_.append(pt)
                pT[i] = p_

            def ffn_out(i):
                for m in range(NCT):
                    wa = wop.tile([128, DFF // 2], BF16, tag="wo", name=f"woA{i}_{m}")
                    nc.sync.dma_start(wa[:], wo_d[m][:, 0:DFF // 2])
                    wb = wop.tile([128, DFF // 2], BF16, tag="wo", name=f"woB{i}_{m}")
                    nc.sync.dma_start(wb[:], wo_d[m][:, DFF // 2:DFF])
                    py = psmm.tile([128, BN], F32, tag="pmm", name=f"py{i}_{m}")
                    for k in range(32):
                        wt = wa if k < 16 else wb
                        ks = (k % 16) * 128
                        nc.tensor.matmul(py[:], wt[:, ks:ks + 128], pT[i][k][:],
                                         start=(k == 0), stop=(k == 31))
                    yo = yp.tile([128, BN], F32, tag="yo", name=f"yo{i}_{m}")
                    nc.vector.tensor_add(yo[:], x2[i][:, m, :], py[:])
                    nc.sync.dma_start(yT_d[:, m, i * BN:(i + 1) * BN], yo[:])

            # ---------------- schedule ----------------
            norm_front(0)
            norm_front(1)
            body_front(0)
            norm_front(2)
            body_front(1)
            down_block(0)
            for i in range(1, NB):
                ffn_norm(i - 1)
                if i + 2 < NB:
                    norm_front(i + 2)
                if i + 1 < NB:
                    body_front(i + 1)
                ffn_gup(i - 1)
                ffn_out(i - 1)
                down_block(i)
            ffn_norm(NB - 1)
            ffn_gup(NB - 1)
            ffn_out(NB - 1)

    if not nc.is_finalized():
        nc.finalize()
    return nc


def _host_prep(x, ln1_w, ln2_w, w_fine, w_medium, w_coarse, sg_w, up_w, down_w, wg, wu, wo):
    import ml_dtypes
    f = np.float32
    bf = ml_dtypes.bfloat16

    def mtiles(w):  # [F, D] -> [F//128, 128, D] stationary tiles
        F_ = w.shape[0]
        return np.ascontiguousarray(
            w.T.reshape(NCT, 128, F_ // 128, 128).transpose(2, 1, 0, 3).reshape(F_ // 128, 128, D))

    def group2(t):  # [M,128,D] -> [M//2,128,2D]
        M = t.shape[0]
        return np.ascontiguousarray(
            t.reshape(M // 2, 2, 128, D).transpose(0, 2, 1, 3).reshape(M // 2, 128, 2 * D))

    up_g = group2(mtiles(up_w)).astype(bf)            # g0..3 gate, g4..7 val
    down_g = group2(mtiles(down_w)).astype(bf)
    wg_g = group2(mtiles(wg)).astype(bf)
    wu_g = group2(mtiles(wu)).astype(bf)
    wo_sb = np.ascontiguousarray(
        wo.T.reshape(32, 128, 8, 128).transpose(2, 1, 0, 3).reshape(8, 128, DFF)).astype(bf)
    sg_p = np.ascontiguousarray(
        sg_w.T.reshape(NCT, 128, 3).transpose(1, 0, 2)).astype(bf)      # [128, NCT, 3]
    ln1_p = np.ascontiguousarray(ln1_w.reshape(NCT, 128).T, f)          # [128, NCT]
    ln2_p = np.ascontiguousarray(ln2_w.reshape(NCT, 128).T, f)
    taps = np.zeros((NCT, 128, 15), f)
    for (w_, nt, base) in ((w_fine, 3, 0), (w_medium, 5, 3), (w_coarse, 7, 8)):
        for j in range(nt):
            taps[:, :, base + j] = w_[:, 0, nt - 1 - j].reshape(NCT, 128)
    taps_p = np.ascontiguousarray(taps.transpose(1, 0, 2))              # [128, NCT, 15]
    shared = dict(up_g=up_g, down_g=down_g, wg_g=wg_g, wu_g=wu_g,
                  wo_sb=wo_sb, sg_p=sg_p, ln1_p=ln1_p, ln2_p=ln2_p,
                  taps_p=taps_p,
                  ones128=np.ones((128, 1), bf), one1=np.ones((1, 128), bf))
    in_maps = []
    for core in range(8):
        b, half = core // 2, core % 2
        if half == 0:
            histx = np.zeros((H, D), f)
            pay = x[b, 0:R]
        else:
            histx = x[b, R - H:R]
            pay = x[b, R:S]
        xcat = np.concatenate([histx, pay], 0)        # [H+R, D]
        xTh = np.ascontiguousarray(
            xcat.reshape(H + R, NCT, 128).transpose(2, 1, 0)).astype(bf)
        in_maps.append({**shared, "xT": xTh})
    return in_maps


def kernel(**inputs):
    from concourse.bass_utils import run_bass_kernel_spmd
    if "nc" not in _CACHE:
        _CACHE["nc"] = _build()
    nc = _CACHE["nc"]
    in_maps = _host_prep(**{k: np.asarray(v) for k, v in inputs.items()})
    res = run_bass_kernel_spmd(nc, in_maps, core_ids=list(range(8)))
    out = np.empty((B, S, D), np.float32)
    for core in range(8):
        b, half = core // 2, core % 2
        yTh = res.results[core]["yT"]                 # [128, NCT, R]
        out[b, half * R:(half + 1) * R] = yTh.transpose(2, 1, 0).reshape(R, D)
    return out



# revision 7
# speedup vs baseline: 1.1698x; 1.1698x over previous
"""CortexIIBlock TRN2 Bass kernel v3 — fp8 DoubleRow 3-pair GEMMs.

8-core data-parallel over (batch, seq-half): each core owns 2048 sequence
positions. All big GEMMs run as fp8(e4m3) DoubleRow matmuls with an
error-compensated 3-pair split: for A ~= Ah+Al (hi + residual, same scale)
and W ~= Wh+Wl, accumulate Wh.Ah + Wh.Al + Wl.Ah in fp32 PSUM. Each
DoubleRow instruction carries two (weights, ifmap) k-pairs at 0.5
cycles/row, so the 3-pair scheme costs 0.75 cycles per 128-K tile per
output column vs 1.0 for bf16 -- a 1.33x PE speedup at ~2e-3 rel error.
Activations are pre-scaled (x16 / x8, folded into existing ops) so fp8
operands sit in e4m3's normal range; descales fold into the post-PSUM
activation/STT scales. Depthwise convs + softmax mixing stay bf16 on
DVE/GpSimd. Causal conv history (16 cols) is precomputed on the host.
"""
import numpy as np

D = 1024
DFF = 4096
B = 4
S = 4096
H = 16           # conv history cols (lookback <= 6, padded to 16)
R = 2048         # payload cols per shard
NCT = D // 128   # 8 channel tiles
NB = 4           # payload blocks
BN = 512
EPS = 1e-6

SA = 16.0        # activation scale for h, z(fused*gate), h2
SAP = 8.0        # activation scale for p = silu(g)*u
SW_UP = 1024.0
SW_DN = 1024.0
SW_G = 1024.0
SW_U = 1024.0
SW_O = 2048.0
SW_SG = 2048.0

_CACHE = {}


def _build():
    import concourse.bacc as bacc
    import concourse.mybir as mybir
    import concourse.tile as tile

    F32 = mybir.dt.float32
    BF16 = mybir.dt.bfloat16
    F8 = mybir.dt.float8e4
    AF = mybir.ActivationFunctionType
    MUL = mybir.AluOpType.mult
    ADD = mybir.AluOpType.add
    SUB = mybir.AluOpType.subtract
    DR = mybir.MatmulPerfMode.DoubleRow

    nc = bacc.Bacc(None, target_bir_lowering=False)
    _lp = nc.allow_low_precision(reason="fp8 3-pair GEMMs within tolerance")
    _lp.__enter__()

    xT_d = nc.dram_tensor("xT", [128, NCT, R], BF16, kind="ExternalInput")
    vh_d = nc.dram_tensor("vh", [128, NCT, H], BF16, kind="ExternalInput")
    up8_d = nc.dram_tensor("up8", [16, 128, 4, 2, 2, 128], F8, kind="ExternalInput")
    dn8_d = nc.dram_tensor("dn8", [8, 128, 4, 2, 2, 128], F8, kind="ExternalInput")
    wg8_d = nc.dram_tensor("wg8", [32, 128, 4, 2, 2, 128], F8, kind="ExternalInput")
    wu8_d = nc.dram_tensor("wu8", [32, 128, 4, 2, 2, 128], F8, kind="ExternalInput")
    wo8_d = nc.dram_tensor("wo8", [8, 4, 128, 4, 2, 2, 128], F8, kind="ExternalInput")
    sg8_d = nc.dram_tensor("sg8", [128, 4, 2, 2, 16], F8, kind="ExternalInput")
    ln1_d = nc.dram_tensor("ln1_p", [128, NCT], F32, kind="ExternalInput")
    ln2_d = nc.dram_tensor("ln2_p", [128, NCT], F32, kind="ExternalInput")
    taps_d = nc.dram_tensor("taps_p", [128, NCT, 15], F32, kind="ExternalInput")
    ones128_d = nc.dram_tensor("ones128", [128, 1], BF16, kind="ExternalInput")
    one1_d = nc.dram_tensor("one1", [1, 128], BF16, kind="ExternalInput")
    yT_d = nc.dram_tensor("yT", [128, NCT, R], F32, kind="ExternalOutput")

    from contextlib import ExitStack
    with tile.TileContext(nc) as tc:
        with ExitStack() as stack:
            ep = stack.enter_context
            cpool = ep(tc.tile_pool(name="const", bufs=1))
            xp = ep(tc.tile_pool(name="xp", bufs=2))
            scr = ep(tc.tile_pool(name="scr", bufs=3))       # h_s / z_s / h2_s
            a8p = ep(tc.tile_pool(name="a8p", bufs=4))       # h8/z8/h28 rotate
            vp = ep(tc.tile_pool(name="vp", bufs=2))
            gp = ep(tc.tile_pool(name="gp", bufs=2))
            x2p = ep(tc.tile_pool(name="x2p", bufs=2))
            cvp = ep(tc.tile_pool(name="cvp", bufs=2))
            psp = ep(tc.tile_pool(name="psp", bufs=2))       # p_s pairs
            p8p = ep(tc.tile_pool(name="p8p", bufs=1))       # 16 pair tags
            tgp = ep(tc.tile_pool(name="tg", bufs=2))
            yp = ep(tc.tile_pool(name="yp", bufs=2))
            sqp = ep(tc.tile_pool(name="sqq", bufs=2))
            smp = ep(tc.tile_pool(name="sm", bufs=2))
            sbp = ep(tc.tile_pool(name="sb", bufs=2))
            wup = ep(tc.tile_pool(name="wup", bufs=2))       # up/down stream
            wgp = ep(tc.tile_pool(name="wgp", bufs=2))       # wg/wu stream
            wop = ep(tc.tile_pool(name="wop", bufs=3))       # wo halves
            psmm = ep(tc.tile_pool(name="psmm", bufs=4, space="PSUM"))
            psbc = ep(tc.tile_pool(name="psbc", bufs=2, space="PSUM"))
            psrd = ep(tc.tile_pool(name="psrd", bufs=2, space="PSUM"))

            # ---------------- constants ----------------
            ones128 = cpool.tile([128, 1], BF16, tag="c_ones", name="c_ones")
            nc.sync.dma_start(ones128[:], ones128_d[:])
            one1 = cpool.tile([1, 128], BF16, tag="c_one1", name="c_one1")
            nc.sync.dma_start(one1[:], one1_d[:])
            eps_t = cpool.tile([1, 1], F32, tag="c_eps", name="c_eps")
            nc.vector.memset(eps_t[:], EPS)
            ln1_t = cpool.tile([128, NCT], F32, tag="c_ln1", name="c_ln1")
            nc.sync.dma_start(ln1_t[:], ln1_d[:])
            ln2_t = cpool.tile([128, NCT], F32, tag="c_ln2", name="c_ln2")
            nc.sync.dma_start(ln2_t[:], ln2_d[:])
            sg8_t = cpool.tile([128, 4, 2, 2, 16], F8, tag="c_sg", name="c_sg")
            nc.sync.dma_start(sg8_t[:], sg8_d[:])
            taps_t = cpool.tile([128, NCT, 15], F32, tag="c_taps", name="c_taps")
            nc.sync.dma_start(taps_t[:], taps_d[:])

            # ---------------- persistent per-block state ----------------
            xb = [None] * NB
            h8 = [None] * NB
            z8 = [None] * NB
            h28 = [None] * NB
            val = [None] * NB
            gate = [None] * NB
            swb = [None] * NB
            x2 = [None] * NB
            p8 = [None] * NB

            def dr_gemm(ps, wt, a8, J, first=True, last=True):
                # wt [128, J, 2, 2, 128]; a8 [128, 2, 2J, BN]
                n, tot = 0, 3 * J
                for (g, hh) in ((0, 0), (0, 1), (1, 0)):
                    for j in range(J):
                        nc.tensor.matmul(
                            ps, wt[:, j, g], a8[:, hh, 2 * j:2 * j + 2, :],
                            start=(first and n == 0), stop=(last and n == tot - 1),
                            perf_mode=DR)
                        n += 1

            def rmsnorm_quant(src, ln_t_, a8_, tag):
                # stats: sq (Act), partition-sum (PE), rstd (Act), bcast (PE+Act)
                msum = psrd.tile([1, BN], F32, tag="msum", name=f"msum_{tag}")
                for c in range(NCT):
                    sq = sqp.tile([128, BN], BF16, tag="sq", name=f"sq_{tag}{c}")
                    nc.scalar.activation(sq[:], src[:, c, :], AF.Square)
                    nc.tensor.matmul(msum[:], ones128[:], sq[:],
                                     start=(c == 0), stop=(c == NCT - 1))
                sd = smp.tile([1, BN], F32, tag="sd", name=f"sd_{tag}", bufs=1)
                nc.scalar.activation(sd[:], msum[:], AF.Ln,
                                     bias=eps_t[:], scale=1.0 / D)
                rstd = smp.tile([1, BN], BF16, tag="rstd", name=f"rstd_{tag}")
                nc.scalar.activation(rstd[:], sd[:], AF.Exp, scale=-0.5)
                rsb_ps = psbc.tile([128, BN], F32, tag="pbc", name=f"rsbp_{tag}")
                nc.tensor.matmul(rsb_ps[:], one1[:], rstd[:], start=True, stop=True)
                rsb = sbp.tile([128, BN], BF16, tag="rsb", name=f"rsb_{tag}", bufs=1)
                nc.scalar.copy(rsb[:], rsb_ps[:])
                hs = scr.tile([128, NCT, BN], BF16, tag="scr", name=f"hs_{tag}")
                for c in range(NCT):
                    nc.vector.scalar_tensor_tensor(
                        out=hs[:, c, :], in0=src[:, c, :],
                        scalar=ln_t_[:, c:c + 1], in1=rsb[:], op0=MUL, op1=MUL)
                # quantize: hi on Act, lo on DVE (same scale; residual is fp8-safe)
                nc.scalar.activation(a8_[:, 0], hs[:], AF.Copy)
                nc.vector.scalar_tensor_tensor(
                    out=a8_[:, 1], in0=hs[:], scalar=1.0, in1=a8_[:, 0],
                    op0=MUL, op1=SUB)

            # ---------------- per-block phases ----------------
            def front(i):
                x_ = xp.tile([128, NCT, BN], BF16, tag="xb", name=f"xb{i}")
                nc.sync.dma_start(x_[:], xT_d[:, :, i * BN:(i + 1) * BN])
                xb[i] = x_
                a8_ = a8p.tile([128, 2, NCT, BN], F8, tag="a8", name=f"h8_{i}")
                h8[i] = a8_
                rmsnorm_quant(x_, ln1_t, a8_, f"m{i}")

            def sgup(i):
                a8_ = h8[i]
                # scale-gate softmax: 3 chains of 12 DoubleRow insts each
                ej = []
                for j in range(3):
                    pj = psrd.tile([1, BN], F32, tag="msum", name=f"psg{i}_{j}")
                    n = 0
                    for (g, hh) in ((0, 0), (0, 1), (1, 0)):
                        for jj in range(4):
                            nc.tensor.matmul(
                                pj[:], sg8_t[:, jj, g, :, j:j + 1],
                                a8_[:, hh, 2 * jj:2 * jj + 2, :],
                                start=(n == 0), stop=(n == 11), perf_mode=DR)
                            n += 1
                    e_ = smp.tile([1, BN], BF16, tag=f"e{j}", name=f"e{i}_{j}", bufs=1)
                    nc.scalar.activation(e_[:], pj[:], AF.Exp, scale=1.0 / (SA * SW_SG))
                    ej.append(e_)
                es = smp.tile([1, BN], BF16, tag="es", name=f"es{i}", bufs=1)
                nc.vector.tensor_add(es[:], ej[0][:], ej[1][:])
                nc.vector.tensor_add(es[:], es[:], ej[2][:])
                erec = smp.tile([1, BN], BF16, tag="erec", name=f"erec{i}", bufs=1)
                nc.vector.reciprocal(erec[:], es[:])
                sw_ = []
                for j in range(3):
                    swj = smp.tile([1, BN], BF16, tag="swj", name=f"swj{i}_{j}", bufs=1)
                    nc.vector.tensor_mul(swj[:], ej[j][:], erec[:])
                    pb_ = psbc.tile([128, BN], F32, tag="pbc", name=f"pswb{i}_{j}")
                    nc.tensor.matmul(pb_[:], one1[:], swj[:], start=True, stop=True)
                    sb_ = sbp.tile([128, BN], BF16, tag=f"swb{j}", name=f"swb{i}_{j}", bufs=1)
                    nc.scalar.copy(sb_[:], pb_[:])
                    sw_.append(sb_)
                swb[i] = sw_

                # val half of up projection (m-tiles 8..15)
                vtiles = []
                for c in range(NCT):
                    v_ = vp.tile([128, H + BN], BF16, tag=f"val{c}", name=f"val{i}_{c}")
                    vtiles.append(v_)
                val[i] = vtiles
                for m in range(NCT):
                    wt = wup.tile([128, 4, 2, 2, 128], F8, tag="wup", name=f"wv{i}_{m}")
                    nc.sync.dma_start(wt[:], up8_d[8 + m])
                    pv = psmm.tile([128, BN], F32, tag="pmm", name=f"pval{i}_{m}")
                    dr_gemm(pv[:], wt, a8_, 4)
                    # val scaled x SA: PSUM/(SA*SW_UP) * SA = PSUM/SW_UP
                    nc.scalar.activation(vtiles[m][:, H:H + BN], pv[:], AF.Copy,
                                         scale=1.0 / SW_UP)
                    if i == 0:
                        nc.sync.dma_start(vtiles[m][:, 0:H], vh_d[:, m, :])
                    else:
                        nc.vector.tensor_copy(vtiles[m][:, 0:H],
                                              val[i - 1][m][:, BN:BN + H])

                # gate half of up projection (m-tiles 0..7)
                g_ = gp.tile([128, NCT, BN], BF16, tag="gate", name=f"gate{i}")
                gate[i] = g_
                for m in range(NCT):
                    wt = wup.tile([128, 4, 2, 2, 128], F8, tag="wup", name=f"wg{i}_{m}")
                    nc.sync.dma_start(wt[:], up8_d[m])
                    pg = psmm.tile([128, BN], F32, tag="pmm", name=f"pgate{i}_{m}")
                    dr_gemm(pg[:], wt, a8_, 4)
                    nc.scalar.activation(g_[:, m, :], pg[:], AF.Sigmoid,
                                         scale=1.0 / (SA * SW_UP))

            def convmix(i):
                # convs on DVE, softmax-weighted mix on GpSimd; steady-state at
                # low priority (gap filler; deadline is down(i)). Block 0 is the
                # prologue critical path: split across engines at normal priority.
                z_ = scr.tile([128, NCT, BN], BF16, tag="scr", name=f"zs{i}")
                sw_ = swb[i]
                g_ = gate[i]
                lowp = None
                if i > 0:
                    lowp = tc.high_priority(offset=-10_000_000)
                    lowp.__enter__()
                for c in range(NCT):
                    conv_eng = nc.vector
                    mix_eng = nc.vector if (i == 0 and c >= 6) else nc.gpsimd
                    v_ = val[i][c]
                    convs = []
                    for (nt, base) in ((3, 0), (5, 3), (7, 8)):
                        ct_ = cvp.tile([128, BN], BF16, tag=f"cv{len(convs)}",
                                       name=f"cv{i}_{c}_{len(convs)}")
                        conv_eng.tensor_scalar_mul(
                            ct_[:], v_[:, H:H + BN], taps_t[:, c, base:base + 1])
                        for j in range(1, nt):
                            conv_eng.scalar_tensor_tensor(
                                out=ct_[:], in0=v_[:, H - j:H - j + BN],
                                scalar=taps_t[:, c, base + j:base + j + 1],
                                in1=ct_[:], op0=MUL, op1=ADD)
                        convs.append(ct_)
                    acc = cvp.tile([128, BN], BF16, tag="acc", name=f"acc{i}_{c}")
                    mix_eng.tensor_mul(acc[:], convs[0][:], sw_[0][:])
                    for j in (1, 2):
                        u_ = cvp.tile([128, BN], BF16, tag="mixu", name=f"mixu{i}_{c}")
                        mix_eng.tensor_mul(u_[:], convs[j][:], sw_[j][:])
                        mix_eng.tensor_add(acc[:], acc[:], u_[:])
                    mix_eng.tensor_mul(z_[:, c, :], acc[:], g_[:, c, :])
                if lowp is not None:
                    lowp.__exit__(None, None, None)
                # quantize z: hi on GpSimd, lo on DVE
                z8_ = a8p.tile([128, 2, NCT, BN], F8, tag="a8", name=f"z8_{i}")
                z8[i] = z8_
                nc.gpsimd.tensor_copy(z8_[:, 0], z_[:])
                nc.vector.scalar_tensor_tensor(
                    out=z8_[:, 1], in0=z_[:], scalar=1.0, in1=z8_[:, 0],
                    op0=MUL, op1=SUB)

            def down(i):
                x2_ = x2p.tile([128, NCT, BN], BF16, tag="x2", name=f"x2_{i}")
                x2[i] = x2_
                for m in range(NCT):
                    wt = wup.tile([128, 4, 2, 2, 128], F8, tag="wup", name=f"wd{i}_{m}")
                    nc.sync.dma_start(wt[:], dn8_d[m])
                    pm = psmm.tile([128, BN], F32, tag="pmm", name=f"pmix{i}_{m}")
                    dr_gemm(pm[:], wt, z8[i], 4)
                    nc.vector.scalar_tensor_tensor(
                        out=x2_[:, m, :], in0=pm[:], scalar=1.0 / (SA * SW_DN),
                        in1=xb[i][:, m, :], op0=MUL, op1=ADD)

            def ffnf(i):
                a8_ = a8p.tile([128, 2, NCT, BN], F8, tag="a8", name=f"h28_{i}")
                h28[i] = a8_
                rmsnorm_quant(x2[i], ln2_t, a8_, f"f{i}")

            def gup(i):
                p8_ = []
                ps_cur = [None]
                for m in range(32):
                    wtg = wgp.tile([128, 4, 2, 2, 128], F8, tag="wg", name=f"wgt{i}_{m}")
                    nc.sync.dma_start(wtg[:], wg8_d[m])
                    pg = psmm.tile([128, BN], F32, tag="pmm", name=f"pg{i}_{m}")
                    dr_gemm(pg[:], wtg, h28[i], 4)
                    tg = tgp.tile([128, BN], BF16, tag="tg", name=f"tg{i}_{m}")
                    nc.scalar.activation(tg[:], pg[:], AF.Silu, scale=1.0 / (SA * SW_G))
                    wtu = wgp.tile([128, 4, 2, 2, 128], F8, tag="wu", name=f"wut{i}_{m}")
                    nc.sync.dma_start(wtu[:], wu8_d[m])
                    pu = psmm.tile([128, BN], F32, tag="pmm", name=f"pu{i}_{m}")
                    dr_gemm(pu[:], wtu, h28[i], 4)
                    q, t = m // 2, m % 2
                    if t == 0:
                        ps_cur[0] = psp.tile([128, 2, BN], BF16, tag="ps",
                                             name=f"ps{i}_{q}")
                    # p scaled x SAP: (PSUM/(SA*SW_U)) * silu(g) * SAP
                    nc.vector.scalar_tensor_tensor(
                        out=ps_cur[0][:, t, :], in0=pu[:],
                        scalar=SAP / (SA * SW_U), in1=tg[:], op0=MUL, op1=MUL)
                    if t == 1:
                        pq = p8p.tile([128, 2, 2, BN], F8, tag=f"p8_{q}",
                                      name=f"p8_{i}_{q}")
                        nc.scalar.activation(pq[:, 0], ps_cur[0][:], AF.Copy)
                        nc.vector.scalar_tensor_tensor(
                            out=pq[:, 1], in0=ps_cur[0][:], scalar=1.0,
                            in1=pq[:, 0], op0=MUL, op1=SUB)
                        p8_.append(pq)
                p8[i] = p8_

            def ffn_out(i):
                for m in range(NCT):
                    wq = []
                    for qq in range(4):
                        w_ = wop.tile([128, 4, 2, 2, 128], F8, tag="wo",
                                      name=f"wo{i}_{m}_{qq}")
                        nc.sync.dma_start(w_[:], wo8_d[m, qq])
                        wq.append(w_)
                    py = psmm.tile([128, BN], F32, tag="pmm", name=f"py{i}_{m}")
                    n = 0
                    for q in range(16):
                        wt = wq[q // 4]
                        jj = q % 4
                        for (g, hh) in ((0, 0), (0, 1), (1, 0)):
                            nc.tensor.matmul(
                                py[:], wt[:, jj, g], p8[i][q][:, hh],
                                start=(n == 0), stop=(n == 47), perf_mode=DR)
                            n += 1
                    yo = yp.tile([128, BN], F32, tag="yo", name=f"yo{i}_{m}")
                    nc.vector.scalar_tensor_tensor(
                        out=yo[:], in0=py[:], scalar=1.0 / (SAP * SW_O),
                        in1=x2[i][:, m, :], op0=MUL, op1=ADD)
                    nc.sync.dma_start(yT_d[:, m, i * BN:(i + 1) * BN], yo[:])

            # ---------------- schedule ----------------
            front(0)
            sgup(0)
            front(1)
            convmix(0)
            down(0)
            ffnf(0)
            sgup(1)
            convmix(1)
            for i in range(NB):
                gup(i)
                ffn_out(i)
                if i + 2 < NB:
                    front(i + 2)
                if i + 1 < NB:
                    down(i + 1)
                    ffnf(i + 1)
                if i + 2 < NB:
                    sgup(i + 2)
                    convmix(i + 2)

    if not nc.is_finalized():
        nc.finalize()
    return nc


def _host_prep(x, ln1_w, ln2_w, w_fine, w_medium, w_coarse, sg_w, up_w, down_w, wg, wu, wo):
    import ml_dtypes
    f = np.float32
    bf = ml_dtypes.bfloat16
    f8 = ml_dtypes.float8_e4m3

    def wsplit_pack(w, sw):
        # w [F, D] -> [F//128, 128, D//256, 2(hi/lo), 2(ktile), 128] fp8
        F_, D_ = w.shape
        ws = np.asarray(w, f) * sw
        hi = np.clip(ws, -240, 240).astype(f8)
        lo = (ws - hi.astype(f)).astype(f8)

        def pack(src):
            a = src.reshape(F_ // 128, 128, D_ // 256, 2, 128)  # m, col, j, t, part
            return a.transpose(0, 4, 2, 3, 1)                   # m, part, j, t, col

        return np.ascontiguousarray(np.stack([pack(hi), pack(lo)], axis=3))

    up8 = wsplit_pack(up_w, SW_UP)          # [16, 128, 4, 2, 2, 128]
    dn8 = wsplit_pack(down_w, SW_DN)        # [8, ...]
    wg8 = wsplit_pack(wg, SW_G)             # [32, ...]
    wu8 = wsplit_pack(wu, SW_U)
    wo8_flat = wsplit_pack(wo, SW_O)        # [8, 128, 16, 2, 2, 128]
    wo8 = np.ascontiguousarray(
        wo8_flat.reshape(8, 128, 4, 4, 2, 2, 128).transpose(0, 2, 1, 3, 4, 5, 6))

    sgs = np.asarray(sg_w, f) * SW_SG       # [3, 1024]
    sgh = np.clip(sgs, -240, 240).astype(f8)
    sgl = (sgs - sgh.astype(f)).astype(f8)

    def sg_pack(src):
        a = np.zeros((1024, 16), src.dtype)
        a[:, :3] = src.T
        a = a.reshape(4, 2, 128, 16)        # j, t, part, col
        return a.transpose(2, 0, 1, 3)      # part, j, t, col

    sg8 = np.ascontiguousarray(np.stack([sg_pack(sgh), sg_pack(sgl)], axis=2))

    ln1_p = np.ascontiguousarray(np.asarray(ln1_w, f).reshape(NCT, 128).T * SA)
    ln2_p = np.ascontiguousarray(np.asarray(ln2_w, f).reshape(NCT, 128).T * SA)
    taps = np.zeros((NCT, 128, 15), f)
    for (w_, nt, base) in ((w_fine, 3, 0), (w_medium, 5, 3), (w_coarse, 7, 8)):
        for j in range(nt):
            taps[:, :, base + j] = np.asarray(w_, f)[:, 0, nt - 1 - j].reshape(NCT, 128)
    taps_p = np.ascontiguousarray(taps.transpose(1, 0, 2))

    shared = dict(up8=up8, dn8=dn8, wg8=wg8, wu8=wu8, wo8=wo8, sg8=sg8,
                  ln1_p=ln1_p, ln2_p=ln2_p, taps_p=taps_p,
                  ones128=np.ones((128, 1), bf), one1=np.ones((1, 128), bf))

    xf = np.asarray(x, f)
    upv = np.asarray(up_w, f)[D:2 * D]      # val half [D, D]
    ln1f = np.asarray(ln1_w, f)
    in_maps = []
    for core in range(8):
        b, half = core // 2, core % 2
        pay = xf[b, half * R:(half + 1) * R]                   # [R, D]
        xTh = np.ascontiguousarray(
            pay.reshape(R, NCT, 128).transpose(2, 1, 0)).astype(bf)
        # host-computed conv history: val of the 16 tokens before this shard
        if half == 0:
            vh16 = np.zeros((H, D), f)
        else:
            hist = xf[b, R - H:R]                              # [H, D]
            ms = np.mean(hist * hist, axis=-1, keepdims=True)
            hh = hist / np.sqrt(ms + EPS) * ln1f
            vh16 = hh @ upv.T                                  # [H, D]
        vhT = np.ascontiguousarray(
            (vh16 * SA).reshape(H, NCT, 128).transpose(2, 1, 0)).astype(bf)
        in_maps.append({**shared, "xT": xTh, "vh": vhT})
    return in_maps


def kernel(**inputs):
    from concourse.bass_utils import run_bass_kernel_spmd
    if "nc" not in _CACHE:
        _CACHE["nc"] = _build()
    nc = _CACHE["nc"]
    in_maps = _host_prep(**{k: np.asarray(v) for k, v in inputs.items()})
    res = run_bass_kernel_spmd(nc, in_maps, core_ids=list(range(8)))
    out = np.empty((B, S, D), np.float32)
    for core in range(8):
        b, half = core // 2, core % 2
        yTh = res.results[core]["yT"]                 # [128, NCT, R]
        out[b, half * R:(half + 1) * R] = yTh.transpose(2, 1, 0).reshape(R, D)
    return out


# revision 9
# speedup vs baseline: 1.2480x; 1.0668x over previous
"""CortexIIBlock TRN2 Bass kernel v4 — fp8 DoubleRow GEMMs, DVE-lean ops.

8-core data-parallel over (batch, seq-half): each core owns 2048 sequence
positions. All big GEMMs run as fp8(e4m3) DoubleRow matmuls with an
error-compensated 3-pair split: for A ~= Ah+Al (hi + residual, same scale)
and W ~= Wh+Wl, accumulate Wh.Ah + Wh.Al + Wl.Ah in fp32 PSUM. Each
DoubleRow instruction carries two (weights, ifmap) k-pairs at 0.5
cycles/row, so the 3-pair scheme costs 0.75 cycles per 128-K tile per
output column vs 1.0 for bf16 -- a 1.33x PE speedup at ~2e-3 rel error.
Activations are pre-scaled (x16 / x8, folded into existing ops) so fp8
operands sit in e4m3's normal range; descales fold into the post-PSUM
activation/STT scales. Depthwise convs + softmax mixing stay bf16 on
DVE/GpSimd. Causal conv history (16 cols) is precomputed on the host.
"""
import numpy as np

D = 1024
DFF = 4096
B = 4
S = 4096
H = 16           # conv history cols (lookback <= 6, padded to 16)
R = 2048         # payload cols per shard
NCT = D // 128   # 8 channel tiles
NB = 4           # payload blocks
BN = 512
EPS = 1e-6

SA = 16.0        # activation scale for h, z(fused*gate), h2
SAP = 8.0        # activation scale for p = silu(g)*u
SW_UP = 1024.0
SW_DN = 1024.0
SW_G = 1024.0
SW_U = 1024.0
SW_O = 2048.0
SW_SG = 2048.0

_CACHE = {}


def _build():
    import concourse.bacc as bacc
    import concourse.mybir as mybir
    import concourse.tile as tile

    F32 = mybir.dt.float32
    BF16 = mybir.dt.bfloat16
    F8 = mybir.dt.float8e4
    AF = mybir.ActivationFunctionType
    MUL = mybir.AluOpType.mult
    ADD = mybir.AluOpType.add
    SUB = mybir.AluOpType.subtract
    DR = mybir.MatmulPerfMode.DoubleRow

    nc = bacc.Bacc(None, target_bir_lowering=False)
    _lp = nc.allow_low_precision(reason="fp8 3-pair GEMMs within tolerance")
    _lp.__enter__()

    xT_d = nc.dram_tensor("xT", [128, NCT, R], BF16, kind="ExternalInput")
    vh_d = nc.dram_tensor("vh", [128, NCT, H], BF16, kind="ExternalInput")
    up8_d = nc.dram_tensor("up8", [16, 128, 4, 2, 2, 128], F8, kind="ExternalInput")
    dn8_d = nc.dram_tensor("dn8", [8, 128, 4, 2, 2, 128], F8, kind="ExternalInput")
    wg8_d = nc.dram_tensor("wg8", [32, 128, 4, 2, 2, 128], F8, kind="ExternalInput")
    wu8_d = nc.dram_tensor("wu8", [32, 128, 4, 2, 2, 128], F8, kind="ExternalInput")
    wo8_d = nc.dram_tensor("wo8", [8, 4, 128, 4, 2, 2, 128], F8, kind="ExternalInput")
    sg8_d = nc.dram_tensor("sg8", [128, 4, 2, 2, 16], F8, kind="ExternalInput")
    taps_d = nc.dram_tensor("taps_p", [128, NCT, 15], F32, kind="ExternalInput")
    ones128_d = nc.dram_tensor("ones128", [128, 1], BF16, kind="ExternalInput")
    one1_d = nc.dram_tensor("one1", [1, 128], BF16, kind="ExternalInput")
    sa1_d = nc.dram_tensor("sa1", [1, 128], BF16, kind="ExternalInput")
    yT_d = nc.dram_tensor("yT", [128, NCT, R], F32, kind="ExternalOutput")

    from contextlib import ExitStack
    with tile.TileContext(nc) as tc:
        with ExitStack() as stack:
            ep = stack.enter_context
            cpool = ep(tc.tile_pool(name="const", bufs=1))
            xp = ep(tc.tile_pool(name="xp", bufs=3))
            scr = ep(tc.tile_pool(name="scr", bufs=3))       # h_s / z_s / h2_s
            a8p = ep(tc.tile_pool(name="a8p", bufs=4))       # h8/z8/h28 rotate
            vp = ep(tc.tile_pool(name="vp", bufs=2))
            gp = ep(tc.tile_pool(name="gp", bufs=2))
            x2p = ep(tc.tile_pool(name="x2p", bufs=2))
            cvp = ep(tc.tile_pool(name="cvp", bufs=2))
            p8p = ep(tc.tile_pool(name="p8p", bufs=1))       # 16 pair tags
            tgp = ep(tc.tile_pool(name="tg", bufs=2))
            yp = ep(tc.tile_pool(name="yp", bufs=2))
            sqp = ep(tc.tile_pool(name="sqq", bufs=2))
            smp = ep(tc.tile_pool(name="sm", bufs=2))
            sbp = ep(tc.tile_pool(name="sb", bufs=2))
            wup = ep(tc.tile_pool(name="wup", bufs=2))       # up/down stream
            wgp = ep(tc.tile_pool(name="wgp", bufs=2))       # wg/wu stream
            wop = ep(tc.tile_pool(name="wop", bufs=3))       # wo halves
            psmm = ep(tc.tile_pool(name="psmm", bufs=4, space="PSUM"))
            psbc = ep(tc.tile_pool(name="psbc", bufs=2, space="PSUM"))
            psrd = ep(tc.tile_pool(name="psrd", bufs=2, space="PSUM"))

            # ---------------- constants ----------------
            ones128 = cpool.tile([128, 1], BF16, tag="c_ones", name="c_ones")
            nc.sync.dma_start(ones128[:], ones128_d[:])
            one1 = cpool.tile([1, 128], BF16, tag="c_one1", name="c_one1")
            nc.sync.dma_start(one1[:], one1_d[:])
            sa1 = cpool.tile([1, 128], BF16, tag="c_sa1", name="c_sa1")
            nc.sync.dma_start(sa1[:], sa1_d[:])
            eps_t = cpool.tile([1, 1], F32, tag="c_eps", name="c_eps")
            nc.vector.memset(eps_t[:], EPS)
            sg8_t = cpool.tile([128, 4, 2, 2, 16], F8, tag="c_sg", name="c_sg")
            nc.sync.dma_start(sg8_t[:], sg8_d[:])
            taps_t = cpool.tile([128, NCT, 15], F32, tag="c_taps", name="c_taps")
            nc.sync.dma_start(taps_t[:], taps_d[:])

            # ---------------- persistent per-block state ----------------
            xb = [None] * NB
            h8 = [None] * NB
            z8 = [None] * NB
            h28 = [None] * NB
            val = [None] * NB
            gate = [None] * NB
            swb = [None] * NB
            x2 = [None] * NB
            p8 = [None] * NB

            def dr_gemm(ps, wt, a8, J):
                # 3-pair, Al-dependent instructions last (lo quant can lag)
                n, tot = 0, 3 * J
                for (g, hh) in ((0, 0), (1, 0), (0, 1)):
                    for j in range(J):
                        nc.tensor.matmul(
                            ps, wt[:, j, g], a8[:, hh, 2 * j:2 * j + 2, :],
                            start=(n == 0), stop=(n == tot - 1), perf_mode=DR)
                        n += 1

            def rmsnorm_quant(src, a8_, tag):
                # stats: sq (Act), partition-sum (PE), rstd (Act), bcast (PE+Act)
                msum = psrd.tile([1, BN], F32, tag="msum", name=f"msum_{tag}")
                for c in range(NCT):
                    sq = sqp.tile([128, BN], BF16, tag="sq", name=f"sq_{tag}{c}")
                    nc.scalar.activation(sq[:], src[:, c, :], AF.Square)
                    nc.tensor.matmul(msum[:], ones128[:], sq[:],
                                     start=(c == 0), stop=(c == NCT - 1))
                sd = smp.tile([1, BN], F32, tag="sd", name=f"sd_{tag}", bufs=1)
                nc.scalar.activation(sd[:], msum[:], AF.Ln,
                                     bias=eps_t[:], scale=1.0 / D)
                rstd = smp.tile([1, BN], BF16, tag="rstd", name=f"rstd_{tag}")
                nc.scalar.activation(rstd[:], sd[:], AF.Exp, scale=-0.5)
                rsb_ps = psbc.tile([128, BN], F32, tag="pbc", name=f"rsbp_{tag}")
                nc.tensor.matmul(rsb_ps[:], sa1[:], rstd[:], start=True, stop=True)
                rsb = sbp.tile([128, BN], BF16, tag="rsb", name=f"rsb_{tag}", bufs=1)
                nc.scalar.copy(rsb[:], rsb_ps[:])
                hs = scr.tile([128, NCT, BN], BF16, tag="scr", name=f"hs_{tag}")
                for c in range(NCT):
                    nc.vector.tensor_mul(hs[:, c, :], src[:, c, :], rsb[:])
                # quantize in halves so GEMM chains can start on half 0
                for hf in range(2):
                    c0, c1 = hf * 4, hf * 4 + 4
                    nc.scalar.activation(a8_[:, 0, c0:c1, :], hs[:, c0:c1, :], AF.Copy)
                for hf in range(2):
                    c0, c1 = hf * 4, hf * 4 + 4
                    nc.vector.scalar_tensor_tensor(
                        out=a8_[:, 1, c0:c1, :], in0=hs[:, c0:c1, :], scalar=1.0,
                        in1=a8_[:, 0, c0:c1, :], op0=MUL, op1=SUB)

            # ---------------- per-block phases ----------------
            def front(i):
                x_ = xp.tile([128, NCT, BN], BF16, tag="xb", name=f"xb{i}")
                nc.sync.dma_start(x_[:], xT_d[:, :, i * BN:(i + 1) * BN])
                xb[i] = x_
                a8_ = a8p.tile([128, 2, NCT, BN], F8, tag="a8", name=f"h8_{i}")
                h8[i] = a8_
                rmsnorm_quant(x_, a8_, f"m{i}")

            def sgup(i):
                a8_ = h8[i]
                # scale-gate softmax: 3 chains of 12 DoubleRow insts each
                ej = []
                for j in range(3):
                    pj = psrd.tile([1, BN], F32, tag="msum", name=f"psg{i}_{j}")
                    n = 0
                    for (g, hh) in ((0, 0), (1, 0), (0, 1)):
                        for jj in range(4):
                            nc.tensor.matmul(
                                pj[:], sg8_t[:, jj, g, :, j:j + 1],
                                a8_[:, hh, 2 * jj:2 * jj + 2, :],
                                start=(n == 0), stop=(n == 11), perf_mode=DR)
                            n += 1
                    e_ = smp.tile([1, BN], BF16, tag=f"e{j}", name=f"e{i}_{j}", bufs=1)
                    nc.scalar.activation(e_[:], pj[:], AF.Exp, scale=1.0 / (SA * SW_SG))
                    ej.append(e_)
                es = smp.tile([1, BN], BF16, tag="es", name=f"es{i}", bufs=1)
                nc.vector.tensor_add(es[:], ej[0][:], ej[1][:])
                nc.vector.tensor_add(es[:], es[:], ej[2][:])
                erec = smp.tile([1, BN], BF16, tag="erec", name=f"erec{i}", bufs=1)
                nc.vector.reciprocal(erec[:], es[:])
                sw_ = []
                for j in range(3):
                    swj = smp.tile([1, BN], BF16, tag="swj", name=f"swj{i}_{j}", bufs=1)
                    nc.vector.tensor_mul(swj[:], ej[j][:], erec[:])
                    pb_ = psbc.tile([128, BN], F32, tag="pbc", name=f"pswb{i}_{j}")
                    nc.tensor.matmul(pb_[:], one1[:], swj[:], start=True, stop=True)
                    sb_ = sbp.tile([128, BN], BF16, tag=f"swb{j}", name=f"swb{i}_{j}", bufs=1)
                    nc.scalar.copy(sb_[:], pb_[:])
                    sw_.append(sb_)
                swb[i] = sw_

                # val half of up projection (m-tiles 8..15)
                vtiles = []
                for c in range(NCT):
                    v_ = vp.tile([128, H + BN], BF16, tag=f"val{c}", name=f"val{i}_{c}")
                    vtiles.append(v_)
                val[i] = vtiles
                for m in range(NCT):
                    wt = wup.tile([128, 4, 2, 2, 128], F8, tag="wup", name=f"wv{i}_{m}")
                    nc.sync.dma_start(wt[:], up8_d[8 + m])
                    pv = psmm.tile([128, BN], F32, tag="pmm", name=f"pval{i}_{m}")
                    dr_gemm(pv[:], wt, a8_, 4)
                    # val scaled x SA: PSUM/(SA*SW_UP) * SA = PSUM/SW_UP
                    nc.scalar.activation(vtiles[m][:, H:H + BN], pv[:], AF.Copy,
                                         scale=1.0 / SW_UP)
                    if i == 0:
                        nc.sync.dma_start(vtiles[m][:, 0:H], vh_d[:, m, :])
                    else:
                        nc.vector.tensor_copy(vtiles[m][:, 0:H],
                                              val[i - 1][m][:, BN:BN + H])

                # gate half of up projection (m-tiles 0..7)
                g_ = gp.tile([128, NCT, BN], BF16, tag="gate", name=f"gate{i}")
                gate[i] = g_
                for m in range(NCT):
                    wt = wup.tile([128, 4, 2, 2, 128], F8, tag="wup", name=f"wg{i}_{m}")
                    nc.sync.dma_start(wt[:], up8_d[m])
                    pg = psmm.tile([128, BN], F32, tag="pmm", name=f"pgate{i}_{m}")
                    dr_gemm(pg[:], wt, a8_, 4)
                    nc.scalar.activation(g_[:, m, :], pg[:], AF.Sigmoid,
                                         scale=1.0 / (SA * SW_UP))

            def convmix(i):
                # convs on DVE, softmax-weighted mix on GpSimd; steady-state at
                # low priority (gap filler; deadline is down(i)). Block 0 is the
                # prologue critical path: split across engines at normal priority.
                z_ = scr.tile([128, NCT, BN], BF16, tag="scr", name=f"zs{i}")
                sw_ = swb[i]
                g_ = gate[i]
                lowp = None
                if i > 0:
                    lowp = tc.high_priority(offset=-10_000_000)
                    lowp.__enter__()
                for c in range(NCT):
                    mix_eng = nc.vector if (i == 0 and c >= 6) else nc.gpsimd
                    v_ = val[i][c]
                    convs = []
                    for (nt, base) in ((3, 0), (5, 3), (7, 8)):
                        b = len(convs)
                        ct_ = cvp.tile([128, BN], BF16, tag=f"cv{b}",
                                       name=f"cv{i}_{c}_{b}")
                        nc.vector.tensor_scalar_mul(
                            ct_[:], v_[:, H:H + BN], taps_t[:, c, base:base + 1])
                        for j in range(1, nt):
                            tm_ = cvp.tile([128, BN], BF16, tag="ctmp",
                                           name=f"ctmp{i}_{c}_{b}_{j}")
                            nc.vector.tensor_scalar_mul(
                                tm_[:], v_[:, H - j:H - j + BN],
                                taps_t[:, c, base + j:base + j + 1])
                            nc.vector.tensor_add(ct_[:], ct_[:], tm_[:])
                        convs.append(ct_)
                    acc = cvp.tile([128, BN], BF16, tag="acc", name=f"acc{i}_{c}")
                    mix_eng.tensor_mul(acc[:], convs[0][:], sw_[0][:])
                    for j in (1, 2):
                        u_ = cvp.tile([128, BN], BF16, tag="mixu", name=f"mixu{i}_{c}")
                        mix_eng.tensor_mul(u_[:], convs[j][:], sw_[j][:])
                        mix_eng.tensor_add(acc[:], acc[:], u_[:])
                    mix_eng.tensor_mul(z_[:, c, :], acc[:], g_[:, c, :])
                if lowp is not None:
                    lowp.__exit__(None, None, None)
                # quantize z fully on GpSimd, in halves
                z8_ = a8p.tile([128, 2, NCT, BN], F8, tag="a8", name=f"z8_{i}")
                z8[i] = z8_
                for hf in range(2):
                    c0, c1 = hf * 4, hf * 4 + 4
                    nc.gpsimd.tensor_copy(z8_[:, 0, c0:c1, :], z_[:, c0:c1, :])
                for hf in range(2):
                    c0, c1 = hf * 4, hf * 4 + 4
                    nc.gpsimd.tensor_sub(z8_[:, 1, c0:c1, :], z_[:, c0:c1, :],
                                         z8_[:, 0, c0:c1, :])

            def down(i):
                x2_ = x2p.tile([128, NCT, BN], BF16, tag="x2", name=f"x2_{i}")
                x2[i] = x2_
                for m in range(NCT):
                    wt = wup.tile([128, 4, 2, 2, 128], F8, tag="wup", name=f"wd{i}_{m}")
                    nc.sync.dma_start(wt[:], dn8_d[m])
                    pm = psmm.tile([128, BN], F32, tag="pmm", name=f"pmix{i}_{m}")
                    dr_gemm(pm[:], wt, z8[i], 4)
                    nc.vector.scalar_tensor_tensor(
                        out=x2_[:, m, :], in0=pm[:], scalar=1.0 / (SA * SW_DN),
                        in1=xb[i][:, m, :], op0=MUL, op1=ADD)

            def ffnf(i):
                a8_ = a8p.tile([128, 2, NCT, BN], F8, tag="a8", name=f"h28_{i}")
                h28[i] = a8_
                rmsnorm_quant(x2[i], a8_, f"f{i}")

            def gup(i):
                p8_ = []
                for m in range(32):
                    wtg = wgp.tile([128, 4, 2, 2, 128], F8, tag="wg", name=f"wgt{i}_{m}")
                    nc.sync.dma_start(wtg[:], wg8_d[m])
                    pg = psmm.tile([128, BN], F32, tag="pmm", name=f"pg{i}_{m}")
                    dr_gemm(pg[:], wtg, h28[i], 4)
                    tg = tgp.tile([128, BN], BF16, tag="tg", name=f"tg{i}_{m}")
                    nc.scalar.activation(tg[:], pg[:], AF.Silu, scale=1.0 / (SA * SW_G))
                    wtu = wgp.tile([128, 4, 2, 2, 128], F8, tag="wu", name=f"wut{i}_{m}")
                    nc.sync.dma_start(wtu[:], wu8_d[m])
                    pu = psmm.tile([128, BN], F32, tag="pmm", name=f"pu{i}_{m}")
                    dr_gemm(pu[:], wtu, h28[i], 4)
                    q, t = m // 2, m % 2
                    if t == 0:
                        pq = p8p.tile([128, 2, BN], F8, tag=f"p8_{q}",
                                      name=f"p8_{i}_{q}")
                        p8_.append(pq)
                    # p8 hi written directly from PSUM: (pu*s)*silu(g) -> fp8
                    nc.vector.scalar_tensor_tensor(
                        out=p8_[q][:, t, :], in0=pu[:],
                        scalar=SAP / (SA * SW_U), in1=tg[:], op0=MUL, op1=MUL)
                p8[i] = p8_

            def ffn_out(i):
                for m in range(NCT):
                    wq = []
                    for qq in range(4):
                        w_ = wop.tile([128, 4, 2, 2, 128], F8, tag="wo",
                                      name=f"wo{i}_{m}_{qq}")
                        nc.sync.dma_start(w_[:], wo8_d[m, qq])
                        wq.append(w_)
                    py = psmm.tile([128, BN], F32, tag="pmm", name=f"py{i}_{m}")
                    n = 0
                    for q in range(16):
                        wt = wq[q // 4]
                        jj = q % 4
                        for g in (0, 1):  # 2-pair: (Woh, P), (Wol, P)
                            nc.tensor.matmul(
                                py[:], wt[:, jj, g], p8[i][q][:],
                                start=(n == 0), stop=(n == 31), perf_mode=DR)
                            n += 1
                    yo = yp.tile([128, BN], F32, tag="yo", name=f"yo{i}_{m}")
                    nc.vector.scalar_tensor_tensor(
                        out=yo[:], in0=py[:], scalar=1.0 / (SAP * SW_O),
                        in1=x2[i][:, m, :], op0=MUL, op1=ADD)
                    nc.sync.dma_start(yT_d[:, m, i * BN:(i + 1) * BN], yo[:])

            # ---------------- schedule ----------------
            front(0)
            sgup(0)
            front(1)
            convmix(0)
            sgup(1)
            front(2)
            down(0)
            ffnf(0)
            convmix(1)
            for i in range(NB):
                gup(i)
                ffn_out(i)
                if i + 3 < NB:
                    front(i + 3)
                if i + 1 < NB:
                    down(i + 1)
                    ffnf(i + 1)
                if i + 2 < NB:
                    sgup(i + 2)
                    convmix(i + 2)

    if not nc.is_finalized():
        nc.finalize()
    return nc


def _host_prep(x, ln1_w, ln2_w, w_fine, w_medium, w_coarse, sg_w, up_w, down_w, wg, wu, wo):
    import ml_dtypes
    f = np.float32
    bf = ml_dtypes.bfloat16
    f8 = ml_dtypes.float8_e4m3

    def wsplit_pack(w, sw):
        # w [F, D] -> [F//128, 128, D//256, 2(hi/lo), 2(ktile), 128] fp8
        F_, D_ = w.shape
        ws = np.asarray(w, f) * sw
        hi = np.clip(ws, -240, 240).astype(f8)
        lo = (ws - hi.astype(f)).astype(f8)

        def pack(src):
            a = src.reshape(F_ // 128, 128, D_ // 256, 2, 128)  # m, col, j, t, part
            return a.transpose(0, 4, 2, 3, 1)                   # m, part, j, t, col

        return np.ascontiguousarray(np.stack([pack(hi), pack(lo)], axis=3))

    # fold the rmsnorm elementwise weights into the matmul weight columns
    ln1f = np.asarray(ln1_w, f)
    ln2f = np.asarray(ln2_w, f)
    up_l = np.asarray(up_w, f) * ln1f[None, :]
    sg_l = np.asarray(sg_w, f) * ln1f[None, :]
    wg_l = np.asarray(wg, f) * ln2f[None, :]
    wu_l = np.asarray(wu, f) * ln2f[None, :]

    up8 = wsplit_pack(up_l, SW_UP)          # [16, 128, 4, 2, 2, 128]
    dn8 = wsplit_pack(np.asarray(down_w, f), SW_DN)
    wg8 = wsplit_pack(wg_l, SW_G)
    wu8 = wsplit_pack(wu_l, SW_U)
    wo8_flat = wsplit_pack(np.asarray(wo, f), SW_O)   # [8, 128, 16, 2, 2, 128]
    wo8 = np.ascontiguousarray(
        wo8_flat.reshape(8, 128, 4, 4, 2, 2, 128).transpose(0, 2, 1, 3, 4, 5, 6))

    sgs = sg_l * SW_SG                      # [3, 1024]
    sgh = np.clip(sgs, -240, 240).astype(f8)
    sgl = (sgs - sgh.astype(f)).astype(f8)

    def sg_pack(src):
        a = np.zeros((1024, 16), src.dtype)
        a[:, :3] = src.T
        a = a.reshape(4, 2, 128, 16)        # j, t, part, col
        return a.transpose(2, 0, 1, 3)      # part, j, t, col

    sg8 = np.ascontiguousarray(np.stack([sg_pack(sgh), sg_pack(sgl)], axis=2))

    taps = np.zeros((NCT, 128, 15), f)
    for (w_, nt, base) in ((w_fine, 3, 0), (w_medium, 5, 3), (w_coarse, 7, 8)):
        for j in range(nt):
            taps[:, :, base + j] = np.asarray(w_, f)[:, 0, nt - 1 - j].reshape(NCT, 128)
    taps_p = np.ascontiguousarray(taps.transpose(1, 0, 2))

    shared = dict(up8=up8, dn8=dn8, wg8=wg8, wu8=wu8, wo8=wo8, sg8=sg8,
                  taps_p=taps_p,
                  ones128=np.ones((128, 1), bf), one1=np.ones((1, 128), bf),
                  sa1=np.full((1, 128), SA, bf))

    xf = np.asarray(x, f)
    upv_l = up_l[D:2 * D]                   # ln-folded val half [D, D]
    in_maps = []
    for core in range(8):
        b, half = core // 2, core % 2
        pay = xf[b, half * R:(half + 1) * R]                   # [R, D]
        xTh = np.ascontiguousarray(
            pay.reshape(R, NCT, 128).transpose(2, 1, 0)).astype(bf)
        # host-computed conv history: val of the 16 tokens before this shard
        if half == 0:
            vh16 = np.zeros((H, D), f)
        else:
            hist = xf[b, R - H:R]                              # [H, D]
            ms = np.mean(hist * hist, axis=-1, keepdims=True)
            hh = hist / np.sqrt(ms + EPS)
            vh16 = hh @ upv_l.T                                # [H, D]
        vhT = np.ascontiguousarray(
            (vh16 * SA).reshape(H, NCT, 128).transpose(2, 1, 0)).astype(bf)
        in_maps.append({**shared, "xT": xTh, "vh": vhT})
    return in_maps


def kernel(**inputs):
    from concourse.bass_utils import run_bass_kernel_spmd
    if "nc" not in _CACHE:
        _CACHE["nc"] = _build()
    nc = _CACHE["nc"]
    in_maps = _host_prep(**{k: np.asarray(v) for k, v in inputs.items()})
    res = run_bass_kernel_spmd(nc, in_maps, core_ids=list(range(8)))
    out = np.empty((B, S, D), np.float32)
    for core in range(8):
        b, half = core // 2, core % 2
        yTh = res.results[core]["yT"]                 # [128, NCT, R]
        out[b, half * R:(half + 1) * R] = yTh.transpose(2, 1, 0).reshape(R, D)
    return out


# revision 11
# speedup vs baseline: 1.3854x; 1.1102x over previous
"""CortexIIBlock TRN2 Bass kernel v4 — fp8 DoubleRow GEMMs, DVE-lean ops.

8-core data-parallel over (batch, seq-half): each core owns 2048 sequence
positions. All big GEMMs run as fp8(e4m3) DoubleRow matmuls with an
error-compensated 3-pair split: for A ~= Ah+Al (hi + residual, same scale)
and W ~= Wh+Wl, accumulate Wh.Ah + Wh.Al + Wl.Ah in fp32 PSUM. Each
DoubleRow instruction carries two (weights, ifmap) k-pairs at 0.5
cycles/row, so the 3-pair scheme costs 0.75 cycles per 128-K tile per
output column vs 1.0 for bf16 -- a 1.33x PE speedup at ~2e-3 rel error.
Activations are pre-scaled (x16 / x8, folded into existing ops) so fp8
operands sit in e4m3's normal range; descales fold into the post-PSUM
activation/STT scales. Depthwise convs + softmax mixing stay bf16 on
DVE/GpSimd. Causal conv history (16 cols) is precomputed on the host.
"""
import numpy as np

D = 1024
DFF = 4096
B = 4
S = 4096
H = 16           # conv history cols (lookback <= 6, padded to 16)
R = 2048         # payload cols per shard
NCT = D // 128   # 8 channel tiles
NB = 4           # payload blocks
BN = 512
EPS = 1e-6

SA = 16.0        # activation scale for h, z(fused*gate), h2
SAP = 8.0        # activation scale for p = silu(g)*u
SW_UP = 1024.0
SW_DN = 1024.0
SW_G = 1024.0
SW_U = 1024.0
SW_O = 2048.0
SW_SG = 2048.0

_CACHE = {}


def _build():
    import concourse.bacc as bacc
    import concourse.mybir as mybir
    import concourse.tile as tile

    F32 = mybir.dt.float32
    BF16 = mybir.dt.bfloat16
    F8 = mybir.dt.float8e4
    AF = mybir.ActivationFunctionType
    MUL = mybir.AluOpType.mult
    ADD = mybir.AluOpType.add
    SUB = mybir.AluOpType.subtract
    DR = mybir.MatmulPerfMode.DoubleRow

    nc = bacc.Bacc(None, target_bir_lowering=False)
    _lp = nc.allow_low_precision(reason="fp8 3-pair GEMMs within tolerance")
    _lp.__enter__()

    xT_d = nc.dram_tensor("xT", [128, NCT, R], BF16, kind="ExternalInput")
    vh_d = nc.dram_tensor("vh", [128, NCT, H], BF16, kind="ExternalInput")
    up8_d = nc.dram_tensor("up8", [16, 128, 4, 2, 2, 128], F8, kind="ExternalInput")
    dn8_d = nc.dram_tensor("dn8", [8, 128, 4, 2, 2, 128], F8, kind="ExternalInput")
    wgu8_d = nc.dram_tensor("wgu8", [32, 128, 2, 4, 2, 2, 128], F8, kind="ExternalInput")
    wo8_d = nc.dram_tensor("wo8", [8, 4, 128, 4, 2, 2, 128], F8, kind="ExternalInput")
    sg8_d = nc.dram_tensor("sg8", [128, 4, 2, 2, 16], F8, kind="ExternalInput")
    taps_d = nc.dram_tensor("taps_p", [128, NCT, 15], F32, kind="ExternalInput")
    ones128_d = nc.dram_tensor("ones128", [128, 1], BF16, kind="ExternalInput")
    one1_d = nc.dram_tensor("one1", [1, 128], BF16, kind="ExternalInput")
    sa1_d = nc.dram_tensor("sa1", [1, 128], BF16, kind="ExternalInput")
    yT_d = nc.dram_tensor("yT", [128, NCT, R], F32, kind="ExternalOutput")

    from contextlib import ExitStack
    with tile.TileContext(nc) as tc:
        with ExitStack() as stack:
            ep = stack.enter_context
            cpool = ep(tc.tile_pool(name="const", bufs=1))
            xp = ep(tc.tile_pool(name="xp", bufs=3))
            scr = ep(tc.tile_pool(name="scr", bufs=3))       # h_s / z_s / h2_s
            a8p = ep(tc.tile_pool(name="a8p", bufs=4))       # h8/z8/h28 rotate
            vp = ep(tc.tile_pool(name="vp", bufs=2))
            gp = ep(tc.tile_pool(name="gp", bufs=2))
            x2p = ep(tc.tile_pool(name="x2p", bufs=2))
            cvp = ep(tc.tile_pool(name="cvp", bufs=2))
            p8p = ep(tc.tile_pool(name="p8p", bufs=1))       # 16 pair tags
            tgp = ep(tc.tile_pool(name="tg", bufs=2))
            yp = ep(tc.tile_pool(name="yp", bufs=2))
            sqp = ep(tc.tile_pool(name="sqq", bufs=2))
            smp = ep(tc.tile_pool(name="sm", bufs=2))
            sbp = ep(tc.tile_pool(name="sb", bufs=2))
            wup = ep(tc.tile_pool(name="wup", bufs=4))       # up/down stream
            wgp = ep(tc.tile_pool(name="wgp", bufs=3))       # wg/wu stream
            wop = ep(tc.tile_pool(name="wop", bufs=4))       # wo halves
            psmm = ep(tc.tile_pool(name="psmm", bufs=4, space="PSUM"))
            psbc = ep(tc.tile_pool(name="psbc", bufs=2, space="PSUM"))
            psrd = ep(tc.tile_pool(name="psrd", bufs=2, space="PSUM"))

            # ---------------- constants ----------------
            ones128 = cpool.tile([128, 1], BF16, tag="c_ones", name="c_ones")
            nc.sync.dma_start(ones128[:], ones128_d[:])
            one1 = cpool.tile([1, 128], BF16, tag="c_one1", name="c_one1")
            nc.sync.dma_start(one1[:], one1_d[:])
            sa1 = cpool.tile([1, 128], BF16, tag="c_sa1", name="c_sa1")
            nc.sync.dma_start(sa1[:], sa1_d[:])
            eps_t = cpool.tile([1, 1], F32, tag="c_eps", name="c_eps")
            nc.vector.memset(eps_t[:], EPS)
            sg8_t = cpool.tile([128, 4, 2, 2, 16], F8, tag="c_sg", name="c_sg")
            nc.sync.dma_start(sg8_t[:], sg8_d[:])
            taps_t = cpool.tile([128, NCT, 15], F32, tag="c_taps", name="c_taps")
            nc.sync.dma_start(taps_t[:], taps_d[:])

            # ---------------- persistent per-block state ----------------
            xb = [None] * NB
            h8 = [None] * NB
            z8 = [None] * NB
            h28 = [None] * NB
            val = [None] * NB
            gate = [None] * NB
            swb = [None] * NB
            x2 = [None] * NB
            p8 = [None] * NB

            def dr_gemm(ps, wt, a8, J, wsel=None):
                # 3-pair, Al-dependent instructions last (lo quant can lag)
                n, tot = 0, 3 * J
                for (g, hh) in ((0, 0), (1, 0), (0, 1)):
                    for j in range(J):
                        wap = (wt[:, j, g] if wsel is None
                               else wt[:, wsel, j, g])
                        nc.tensor.matmul(
                            ps, wap, a8[:, hh, 2 * j:2 * j + 2, :],
                            start=(n == 0), stop=(n == tot - 1), perf_mode=DR)
                        n += 1

            def rmsnorm_quant(src, a8_, tag):
                # stats: sq (Act), partition-sum (PE), rstd (Act), bcast (PE+Act)
                msum = psrd.tile([1, BN], F32, tag="msum", name=f"msum_{tag}")
                for c in range(NCT):
                    sq = sqp.tile([128, BN], BF16, tag="sq", name=f"sq_{tag}{c}")
                    nc.scalar.activation(sq[:], src[:, c, :], AF.Square)
                    nc.tensor.matmul(msum[:], ones128[:], sq[:],
                                     start=(c == 0), stop=(c == NCT - 1))
                sd = smp.tile([1, BN], F32, tag="sd", name=f"sd_{tag}", bufs=1)
                nc.scalar.activation(sd[:], msum[:], AF.Ln,
                                     bias=eps_t[:], scale=1.0 / D)
                rstd = smp.tile([1, BN], BF16, tag="rstd", name=f"rstd_{tag}")
                nc.scalar.activation(rstd[:], sd[:], AF.Exp, scale=-0.5)
                rsb_ps = psbc.tile([128, BN], F32, tag="pbc", name=f"rsbp_{tag}")
                nc.tensor.matmul(rsb_ps[:], sa1[:], rstd[:], start=True, stop=True)
                rsb = sbp.tile([128, BN], BF16, tag="rsb", name=f"rsb_{tag}", bufs=1)
                nc.scalar.copy(rsb[:], rsb_ps[:])
                hs = scr.tile([128, NCT, BN], BF16, tag="scr", name=f"hs_{tag}")
                for c in range(NCT):
                    nc.vector.tensor_mul(hs[:, c, :], src[:, c, :], rsb[:])
                # quantize in halves so GEMM chains can start on half 0
                for hf in range(2):
                    c0, c1 = hf * 4, hf * 4 + 4
                    nc.scalar.activation(a8_[:, 0, c0:c1, :], hs[:, c0:c1, :], AF.Copy)
                for hf in range(2):
                    c0, c1 = hf * 4, hf * 4 + 4
                    nc.vector.scalar_tensor_tensor(
                        out=a8_[:, 1, c0:c1, :], in0=hs[:, c0:c1, :], scalar=1.0,
                        in1=a8_[:, 0, c0:c1, :], op0=MUL, op1=SUB)

            # ---------------- per-block phases ----------------
            def front(i):
                x_ = xp.tile([128, NCT, BN], BF16, tag="xb", name=f"xb{i}")
                nc.sync.dma_start(x_[:], xT_d[:, :, i * BN:(i + 1) * BN])
                xb[i] = x_
                a8_ = a8p.tile([128, 2, NCT, BN], F8, tag="a8", name=f"h8_{i}")
                h8[i] = a8_
                rmsnorm_quant(x_, a8_, f"m{i}")

            def sgup(i):
                a8_ = h8[i]
                # scale-gate softmax: 3 chains of 12 DoubleRow insts each
                ej = []
                for j in range(3):
                    pj = psrd.tile([1, BN], F32, tag="msum", name=f"psg{i}_{j}")
                    n = 0
                    for (g, hh) in ((0, 0), (1, 0), (0, 1)):
                        for jj in range(4):
                            nc.tensor.matmul(
                                pj[:], sg8_t[:, jj, g, :, j:j + 1],
                                a8_[:, hh, 2 * jj:2 * jj + 2, :],
                                start=(n == 0), stop=(n == 11), perf_mode=DR)
                            n += 1
                    e_ = smp.tile([1, BN], BF16, tag=f"e{j}", name=f"e{i}_{j}", bufs=1)
                    nc.scalar.activation(e_[:], pj[:], AF.Exp, scale=1.0 / (SA * SW_SG))
                    ej.append(e_)
                es = smp.tile([1, BN], BF16, tag="es", name=f"es{i}", bufs=1)
                nc.vector.tensor_add(es[:], ej[0][:], ej[1][:])
                nc.vector.tensor_add(es[:], es[:], ej[2][:])
                erec = smp.tile([1, BN], BF16, tag="erec", name=f"erec{i}", bufs=1)
                nc.vector.reciprocal(erec[:], es[:])
                sw_ = []
                for j in range(3):
                    swj = smp.tile([1, BN], BF16, tag="swj", name=f"swj{i}_{j}", bufs=1)
                    nc.vector.tensor_mul(swj[:], ej[j][:], erec[:])
                    pb_ = psbc.tile([128, BN], F32, tag="pbc", name=f"pswb{i}_{j}")
                    nc.tensor.matmul(pb_[:], one1[:], swj[:], start=True, stop=True)
                    sb_ = sbp.tile([128, BN], BF16, tag=f"swb{j}", name=f"swb{i}_{j}", bufs=1)
                    nc.scalar.copy(sb_[:], pb_[:])
                    sw_.append(sb_)
                swb[i] = sw_

                # val half of up projection (m-tiles 8..15)
                vtiles = []
                for c in range(NCT):
                    v_ = vp.tile([128, H + BN], BF16, tag=f"val{c}", name=f"val{i}_{c}")
                    vtiles.append(v_)
                val[i] = vtiles
                for m in range(NCT):
                    wt = wup.tile([128, 4, 2, 2, 128], F8, tag="wup", name=f"wv{i}_{m}")
                    nc.sync.dma_start(wt[:], up8_d[8 + m])
                    pv = psmm.tile([128, BN], F32, tag="pmm", name=f"pval{i}_{m}")
                    dr_gemm(pv[:], wt, a8_, 4)
                    # val scaled x SA: PSUM/(SA*SW_UP) * SA = PSUM/SW_UP
                    nc.scalar.activation(vtiles[m][:, H:H + BN], pv[:], AF.Copy,
                                         scale=1.0 / SW_UP)
                    if i == 0:
                        nc.sync.dma_start(vtiles[m][:, 0:H], vh_d[:, m, :])
                    else:
                        nc.vector.tensor_copy(vtiles[m][:, 0:H],
                                              val[i - 1][m][:, BN:BN + H])

                # gate half of up projection (m-tiles 0..7)
                g_ = gp.tile([128, NCT, BN], BF16, tag="gate", name=f"gate{i}")
                gate[i] = g_
                for m in range(NCT):
                    wt = wup.tile([128, 4, 2, 2, 128], F8, tag="wup", name=f"wg{i}_{m}")
                    nc.sync.dma_start(wt[:], up8_d[m])
                    pg = psmm.tile([128, BN], F32, tag="pmm", name=f"pgate{i}_{m}")
                    dr_gemm(pg[:], wt, a8_, 4)
                    nc.scalar.activation(g_[:, m, :], pg[:], AF.Sigmoid,
                                         scale=1.0 / (SA * SW_UP))

            def convmix(i):
                # convs on DVE, softmax-weighted mix on GpSimd; steady-state at
                # low priority (gap filler; deadline is down(i)). Block 0 is the
                # prologue critical path: split across engines at normal priority.
                z_ = scr.tile([128, NCT, BN], BF16, tag="scr", name=f"zs{i}")
                sw_ = swb[i]
                g_ = gate[i]
                lowp = None
                if i > 0:
                    lowp = tc.high_priority(offset=-10_000_000)
                    lowp.__enter__()
                for c in range(NCT):
                    mix_eng = nc.vector if (i == 0 and c >= 6) else nc.gpsimd
                    v_ = val[i][c]
                    convs = []
                    for (nt, base) in ((3, 0), (5, 3), (7, 8)):
                        b = len(convs)
                        ct_ = cvp.tile([128, BN], BF16, tag=f"cv{b}",
                                       name=f"cv{i}_{c}_{b}")
                        nc.vector.tensor_scalar_mul(
                            ct_[:], v_[:, H:H + BN], taps_t[:, c, base:base + 1])
                        for j in range(1, nt):
                            tm_ = cvp.tile([128, BN], BF16, tag="ctmp",
                                           name=f"ctmp{i}_{c}_{b}_{j}")
                            nc.vector.tensor_scalar_mul(
                                tm_[:], v_[:, H - j:H - j + BN],
                                taps_t[:, c, base + j:base + j + 1])
                            nc.vector.tensor_add(ct_[:], ct_[:], tm_[:])
                        convs.append(ct_)
                    acc = cvp.tile([128, BN], BF16, tag="acc", name=f"acc{i}_{c}")
                    mix_eng.tensor_mul(acc[:], convs[0][:], sw_[0][:])
                    for j in (1, 2):
                        u_ = cvp.tile([128, BN], BF16, tag="mixu", name=f"mixu{i}_{c}")
                        mix_eng.tensor_mul(u_[:], convs[j][:], sw_[j][:])
                        mix_eng.tensor_add(acc[:], acc[:], u_[:])
                    mix_eng.tensor_mul(z_[:, c, :], acc[:], g_[:, c, :])
                if lowp is not None:
                    lowp.__exit__(None, None, None)
                # quantize z fully on GpSimd, in halves
                z8_ = a8p.tile([128, 2, NCT, BN], F8, tag="a8", name=f"z8_{i}")
                z8[i] = z8_
                for hf in range(2):
                    c0, c1 = hf * 4, hf * 4 + 4
                    nc.gpsimd.tensor_copy(z8_[:, 0, c0:c1, :], z_[:, c0:c1, :])
                for hf in range(2):
                    c0, c1 = hf * 4, hf * 4 + 4
                    nc.gpsimd.tensor_sub(z8_[:, 1, c0:c1, :], z_[:, c0:c1, :],
                                         z8_[:, 0, c0:c1, :])

            def down(i):
                x2_ = x2p.tile([128, NCT, BN], BF16, tag="x2", name=f"x2_{i}")
                x2[i] = x2_
                for m in range(NCT):
                    wt = wup.tile([128, 4, 2, 2, 128], F8, tag="wup", name=f"wd{i}_{m}")
                    nc.sync.dma_start(wt[:], dn8_d[m])
                    pm = psmm.tile([128, BN], F32, tag="pmm", name=f"pmix{i}_{m}")
                    dr_gemm(pm[:], wt, z8[i], 4)
                    nc.vector.scalar_tensor_tensor(
                        out=x2_[:, m, :], in0=pm[:], scalar=1.0 / (SA * SW_DN),
                        in1=xb[i][:, m, :], op0=MUL, op1=ADD)

            def ffnf(i):
                a8_ = a8p.tile([128, 2, NCT, BN], F8, tag="a8", name=f"h28_{i}")
                h28[i] = a8_
                rmsnorm_quant(x2[i], a8_, f"f{i}")

            def gup(i):
                p8_ = []
                for m in range(32):
                    wt = wgp.tile([128, 2, 4, 2, 2, 128], F8, tag="wgu", name=f"wgu{i}_{m}")
                    nc.sync.dma_start(wt[:], wgu8_d[m])
                    pg = psmm.tile([128, BN], F32, tag="pmm", name=f"pg{i}_{m}")
                    dr_gemm(pg[:], wt, h28[i], 4, wsel=0)
                    tg = tgp.tile([128, BN], BF16, tag="tg", name=f"tg{i}_{m}")
                    nc.scalar.activation(tg[:], pg[:], AF.Silu, scale=1.0 / (SA * SW_G))
                    pu = psmm.tile([128, BN], F32, tag="pmm", name=f"pu{i}_{m}")
                    dr_gemm(pu[:], wt, h28[i], 4, wsel=1)
                    q, t = m // 2, m % 2
                    if t == 0:
                        pq = p8p.tile([128, 2, BN], F8, tag=f"p8_{q}",
                                      name=f"p8_{i}_{q}")
                        p8_.append(pq)
                    # p8 hi written directly from PSUM: (pu*s)*silu(g) -> fp8
                    nc.vector.scalar_tensor_tensor(
                        out=p8_[q][:, t, :], in0=pu[:],
                        scalar=SAP / (SA * SW_U), in1=tg[:], op0=MUL, op1=MUL)
                p8[i] = p8_

            def ffn_out(i):
                for m in range(NCT):
                    wq = []
                    for qq in range(4):
                        w_ = wop.tile([128, 4, 2, 2, 128], F8, tag="wo",
                                      name=f"wo{i}_{m}_{qq}")
                        nc.sync.dma_start(w_[:], wo8_d[m, qq])
                        wq.append(w_)
                    py = psmm.tile([128, BN], F32, tag="pmm", name=f"py{i}_{m}")
                    n = 0
                    for q in range(16):
                        wt = wq[q // 4]
                        jj = q % 4
                        for g in (0, 1):  # 2-pair: (Woh, P), (Wol, P)
                            nc.tensor.matmul(
                                py[:], wt[:, jj, g], p8[i][q][:],
                                start=(n == 0), stop=(n == 31), perf_mode=DR)
                            n += 1
                    yo = yp.tile([128, BN], F32, tag="yo", name=f"yo{i}_{m}")
                    nc.vector.scalar_tensor_tensor(
                        out=yo[:], in0=py[:], scalar=1.0 / (SAP * SW_O),
                        in1=x2[i][:, m, :], op0=MUL, op1=ADD)
                    nc.sync.dma_start(yT_d[:, m, i * BN:(i + 1) * BN], yo[:])

            # ---------------- schedule ----------------
            front(0)
            sgup(0)
            front(1)
            convmix(0)
            sgup(1)
            front(2)
            down(0)
            ffnf(0)
            convmix(1)
            for i in range(NB):
                gup(i)
                ffn_out(i)
                if i + 3 < NB:
                    front(i + 3)
                if i + 1 < NB:
                    down(i + 1)
                    ffnf(i + 1)
                if i + 2 < NB:
                    sgup(i + 2)
                    convmix(i + 2)

    if not nc.is_finalized():
        nc.finalize()
    return nc


def _host_prep(x, ln1_w, ln2_w, w_fine, w_medium, w_coarse, sg_w, up_w, down_w, wg, wu, wo):
    import ml_dtypes
    f = np.float32
    bf = ml_dtypes.bfloat16
    f8 = ml_dtypes.float8_e4m3

    def wsplit_pack(w, sw):
        # w [F, D] -> [F//128, 128, D//256, 2(hi/lo), 2(ktile), 128] fp8
        F_, D_ = w.shape
        ws = np.asarray(w, f) * sw
        hi = np.clip(ws, -240, 240).astype(f8)
        lo = (ws - hi.astype(f)).astype(f8)

        def pack(src):
            a = src.reshape(F_ // 128, 128, D_ // 256, 2, 128)  # m, col, j, t, part
            return a.transpose(0, 4, 2, 3, 1)                   # m, part, j, t, col

        return np.ascontiguousarray(np.stack([pack(hi), pack(lo)], axis=3))

    # fold the rmsnorm elementwise weights into the matmul weight columns
    ln1f = np.asarray(ln1_w, f)
    ln2f = np.asarray(ln2_w, f)
    up_l = np.asarray(up_w, f) * ln1f[None, :]
    sg_l = np.asarray(sg_w, f) * ln1f[None, :]
    wg_l = np.asarray(wg, f) * ln2f[None, :]
    wu_l = np.asarray(wu, f) * ln2f[None, :]

    up8 = wsplit_pack(up_l, SW_UP)          # [16, 128, 4, 2, 2, 128]
    dn8 = wsplit_pack(np.asarray(down_w, f), SW_DN)
    wgu8 = np.ascontiguousarray(np.stack(
        [wsplit_pack(wg_l, SW_G), wsplit_pack(wu_l, SW_U)], axis=2))
    wo8_flat = wsplit_pack(np.asarray(wo, f), SW_O)   # [8, 128, 16, 2, 2, 128]
    wo8 = np.ascontiguousarray(
        wo8_flat.reshape(8, 128, 4, 4, 2, 2, 128).transpose(0, 2, 1, 3, 4, 5, 6))

    sgs = sg_l * SW_SG                      # [3, 1024]
    sgh = np.clip(sgs, -240, 240).astype(f8)
    sgl = (sgs - sgh.astype(f)).astype(f8)

    def sg_pack(src):
        a = np.zeros((1024, 16), src.dtype)
        a[:, :3] = src.T
        a = a.reshape(4, 2, 128, 16)        # j, t, part, col
        return a.transpose(2, 0, 1, 3)      # part, j, t, col

    sg8 = np.ascontiguousarray(np.stack([sg_pack(sgh), sg_pack(sgl)], axis=2))

    taps = np.zeros((NCT, 128, 15), f)
    for (w_, nt, base) in ((w_fine, 3, 0), (w_medium, 5, 3), (w_coarse, 7, 8)):
        for j in range(nt):
            taps[:, :, base + j] = np.asarray(w_, f)[:, 0, nt - 1 - j].reshape(NCT, 128)
    taps_p = np.ascontiguousarray(taps.transpose(1, 0, 2))

    shared = dict(up8=up8, dn8=dn8, wgu8=wgu8, wo8=wo8, sg8=sg8,
                  taps_p=taps_p,
                  ones128=np.ones((128, 1), bf), one1=np.ones((1, 128), bf),
                  sa1=np.full((1, 128), SA, bf))

    xf = np.asarray(x, f)
    upv_l = up_l[D:2 * D]                   # ln-folded val half [D, D]
    in_maps = []
    for core in range(8):
        b, half = core // 2, core % 2
        pay = xf[b, half * R:(half + 1) * R]                   # [R, D]
        xTh = np.ascontiguousarray(
            pay.reshape(R, NCT, 128).transpose(2, 1, 0)).astype(bf)
        # host-computed conv history: val of the 16 tokens before this shard
        if half == 0:
            vh16 = np.zeros((H, D), f)
        else:
            hist = xf[b, R - H:R]                              # [H, D]
            ms = np.mean(hist * hist, axis=-1, keepdims=True)
            hh = hist / np.sqrt(ms + EPS)
            vh16 = hh @ upv_l.T                                # [H, D]
        vhT = np.ascontiguousarray(
            (vh16 * SA).reshape(H, NCT, 128).transpose(2, 1, 0)).astype(bf)
        in_maps.append({**shared, "xT": xTh, "vh": vhT})
    return in_maps


def kernel(**inputs):
    from concourse.bass_utils import run_bass_kernel_spmd
    if "nc" not in _CACHE:
        _CACHE["nc"] = _build()
    nc = _CACHE["nc"]
    in_maps = _host_prep(**{k: np.asarray(v) for k, v in inputs.items()})
    res = run_bass_kernel_spmd(nc, in_maps, core_ids=list(range(8)))
    out = np.empty((B, S, D), np.float32)
    for core in range(8):
        b, half = core // 2, core % 2
        yTh = res.results[core]["yT"]                 # [128, NCT, R]
        out[b, half * R:(half + 1) * R] = yTh.transpose(2, 1, 0).reshape(R, D)
    return out
